# revision 19
# baseline (speedup 1.0000x reference)
"""Trainium2 Bass kernel for nn_EncoderWithClassifier (4-layer encoder + classifier).

Sharding: 8 cores, core c handles (batch b=c//2, sequence half th=c%2, 1024 tokens).
Canonical activation layout: x^T [C=256 (2 chunks of 128 partitions), T_local=1024].

Attention (per 128-token t-tile): scores S [s_tile=128, 8 heads x 128 t] are built
with bf16 K=32 row-packed matmuls, exp'd in one Act op straight to bf16 SBUF, and
o is accumulated UNTRANSPOSED as out[t,d] = expS^T @ V_aug where V_aug carries a
ones column so the softmax denominator falls out of the same matmul (no separate
rowsum matmuls).  o is normalized during PSUM evacuation (per-head [P,1] scalars)
and transposed back to [c,t] with SBUF->SBUF DMA transposes (2-byte xbar path),
keeping PE/DVE free.

All heavy matmuls run 1 cycle/row in the PE: bf16 for attention/proj/FFN weights,
float32r (bitcast) for fp32 LayerNorm statistics.  LN is h = x*A - B with
A = g (x) rstd and B = g (x) (mu*rstd) - b built by outer-product matmuls; rstd
is a bit-hack + Newton rsqrt on the DVE so the Act engine runs EXP ONLY (a single
act-table load for the whole program - table switches would cost 1.3us each).

The Act engine is the roofline (B*H*T^2/8/128 exp elems/core, ~133us/layer); all
other work is hidden under it: post-attention work (proj, LN2, FFN, next-layer
LN1/QKV) is emitted as generators whose chunks are interleaved ("pumped") into
the attention st-loop, and each layer's s-tiles are scheduled local-A-quarter
first / remote-half last so next-layer scores never wait on the (overlapped)
AllGather of h^T (bf16) between the core pair.

PSUM (8 banks): S [128,1024]f32 x2 (4), o_acc [128,8,33->64]f32 x2 (2),
mm [128,512]f32 x2 (2).
"""
import numpy as np
import ml_dtypes

import concourse.bacc as bacc
import concourse.mybir as mybir
import concourse.tile as tile
from concourse import bass_utils, library_config
from concourse.masks import make_identity

V, C, TMAX, H, L = 32000, 256, 2048, 8, 4
HS, FFN = 32, 256
CLS_H, NOUT = 512, 10
B, T = 4, 2048
TL = 1024          # tokens per core
P = 128
NT = TL // P       # 8 local t-tiles
NS = T // P        # 16 s-tiles
EPS = 1e-5
SCALE = C ** (-0.5)
N_CORES = 8
dt = mybir.dt
F32 = dt.float32
F32R = dt.float32r
BF16 = dt.bfloat16
I32 = dt.int32
Alu = mybir.AluOpType
Act = mybir.ActivationFunctionType
X_AXIS = mybir.AxisListType.X
BF = np.dtype(ml_dtypes.bfloat16)

_CACHE = {}
_MARKS = []


def _r(ap):
    """View an fp32 AP as float32r: 1 PE cycle/row (vs 4) at moving dim >=256."""
    return ap.bitcast(F32R)


def _build_program(sim=False):
    nc = bacc.Bacc("TRN2", target_bir_lowering=False, debug=False,
                   num_devices=1 if sim else N_CORES)

    # ---------------- dram I/O ----------------
    tok = nc.dram_tensor("tok", [V, C], BF16, kind="ExternalInput")
    idxw = nc.dram_tensor("idxw", [P, TL // 16], dt.int16, kind="ExternalInput")
    posr = nc.dram_tensor("posr", [P, 2, TL], F32, kind="ExternalInput")
    remidx = nc.dram_tensor("remidx", [P, (2 * P) // 16], dt.int16,
                            kind="ExternalInput")
    wq_d = nc.dram_tensor("wq", [L, P, 2, C], BF16, kind="ExternalInput")
    wk_d = nc.dram_tensor("wk", [L, P, 2, C], BF16, kind="ExternalInput")
    wv_d = nc.dram_tensor("wv", [L, P, 2, C], BF16, kind="ExternalInput")
    wp_d = nc.dram_tensor("wp", [L, P, 2, C], BF16, kind="ExternalInput")
    w1_d = nc.dram_tensor("w1", [L, P, 2, FFN], BF16, kind="ExternalInput")
    w2_d = nc.dram_tensor("w2", [L, P, 2, C], BF16, kind="ExternalInput")
    vecs_d = nc.dram_tensor("vecs", [L, P, 7, 2], F32, kind="ExternalInput")
    # vecs rows: 0 ln1_g, 1 ln1_b, 2 ln2_g, 3 ln2_b, 4 bproj, 5 b1, 6 b2
    gt_d = nc.dram_tensor("gt", [1, 2 * L + 1, C], F32, kind="ExternalInput")
    # gt rows: 2l = ln1_g(l), 2l+1 = ln2_g(l), 2L = lnf_g   (row-vector layout)
    lnf_d = nc.dram_tensor("lnf", [P, 2, 2], F32, kind="ExternalInput")   # g, b
    wc1_d = nc.dram_tensor("wc1", [P, 2, CLS_H], F32, kind="ExternalInput")
    bc1_d = nc.dram_tensor("bc1", [P, CLS_H // P], F32, kind="ExternalInput")
    wc2_d = nc.dram_tensor("wc2", [P, CLS_H // P, NOUT], F32, kind="ExternalInput")
    bc2_d = nc.dram_tensor("bc2", [1, NOUT], F32, kind="ExternalInput")
    out_d = nc.dram_tensor("probs", [1, NOUT], F32, kind="ExternalOutput")

    REPL = [[0, 1], [2, 3], [4, 5], [6, 7]]

    with tile.TileContext(nc) as tc:
        with (
            tc.tile_pool(name="const", bufs=1) as cp,
            tc.tile_pool(name="work", bufs=1) as wk,
            tc.tile_pool(name="exp", bufs=4) as ep,
            tc.tile_pool(name="small", bufs=2) as sp,
            tc.tile_pool(name="osb", bufs=3) as op,
            tc.tile_pool(name="ps", bufs=2, space="PSUM") as ps,
            tc.tile_pool(name="dram", bufs=2, space="DRAM") as dp,
        ):
            nc.gpsimd.load_library(library_config.mlp)

            # ---------------- constants / weights to SBUF ----------------
            inv256 = cp.tile([P, 1], F32, tag="inv256")
            nc.vector.memset(inv256[:], 1.0 / C)

            def load_const(name, dram_ap, shape, dtype=F32):
                t = cp.tile(shape, dtype, tag=name, name=name)
                nc.sync.dma_start(t[:], dram_ap)
                return t

            wq = [load_const(f"wq{l}", wq_d[l], [P, 2, C], BF16) for l in range(L)]
            wkt = [load_const(f"wk{l}", wk_d[l], [P, 2, C], BF16) for l in range(L)]
            wv = [load_const(f"wv{l}", wv_d[l], [P, 2, C], BF16) for l in range(L)]
            wp = [load_const(f"wp{l}", wp_d[l], [P, 2, C], BF16) for l in range(L)]
            w1 = [load_const(f"w1{l}", w1_d[l], [P, 2, FFN], BF16) for l in range(L)]
            w2 = [load_const(f"w2{l}", w2_d[l], [P, 2, C], BF16) for l in range(L)]
            vecs = [load_const(f"vec{l}", vecs_d[l], [P, 7, 2]) for l in range(L)]
            gt = load_const("gt", gt_d[:], [1, 2 * L + 1, C])
            lnf = load_const("lnf", lnf_d[:], [P, 2, 2])
            wc1 = load_const("wc1", wc1_d[:], [P, 2, CLS_H])
            bc1 = load_const("bc1", bc1_d[:], [P, CLS_H // P])
            wc2 = load_const("wc2", wc2_d[:], [P, CLS_H // P, NOUT])
            bc2 = load_const("bc2", bc2_d[:], [1, NOUT])
            idx_sb = load_const("idx_sb", idxw[:], [P, TL // 16], dt.int16)
            remidx_sb = load_const("remidx_sb", remidx[:], [P, (2 * P) // 16],
                                   dt.int16)

            def vap(l, row, cc):
                return vecs[l][:, row, cc:cc + 1]

            # persistent activations
            xT = [wk.tile([P, TL], F32, tag=f"xT{cc}", name=f"xT{cc}")
                  for cc in range(2)]
            oT_sb = wk.tile([P, 2, TL], BF16, tag="oT", name="oT_sb")
            fT = [wk.tile([P, TL], BF16, tag=f"fT{ff}", name=f"fT{ff}")
                  for ff in range(2)]
            h2T = [wk.tile([P, TL], BF16, tag=f"h2T{cc}", name=f"h2T{cc}")
                   for cc in range(2)]
            xf = [wk.tile([P, TL], F32, tag=f"xf{cc}", name=f"xf{cc}")
                  for cc in range(2)]
            emb2 = sp.tile([P, 2, 2], F32, tag="emb2", bufs=1)  # [t-half, cc]

            # ---------------- layernorm (per t-half) ----------------
            def emit_ln_half(src, ln_id, b_of, out, hf):
                """out[cc][:, half] = LN(src)[:, half]; h = x*A - B with
                A = g(x)rstd, B = g(x)(mu*rstd) - b via outer-product matmuls.
                rstd is computed on the DVE (bit-hack + Newton) so Act stays
                exp-only.  Yields between chunks for co-emission."""
                sl = slice(hf * 512, hf * 512 + 512)
                xsq = sp.tile([P, 2, 512], F32, tag="lnsq", name="lnsq")
                for cc in range(2):
                    nc.gpsimd.tensor_mul(xsq[:, cc, :], src[cc][:, sl],
                                         src[cc][:, sl])
                yield
                mu_ps = ps.tile([1, 512], F32, tag="mm", name="mu_ps")
                for kc in range(2):
                    nc.tensor.matmul(mu_ps[:], lhsT=_r(inv256[:]),
                                     rhs=_r(src[kc][:, sl]),
                                     start=(kc == 0), stop=(kc == 1))
                msq_ps = ps.tile([1, 512], F32, tag="mm", name="msq_ps")
                for cc in range(2):
                    nc.tensor.matmul(msq_ps[:], lhsT=_r(inv256[:]),
                                     rhs=_r(xsq[:, cc, :]),
                                     start=(cc == 0), stop=(cc == 1))
                yield
                stA = sp.tile([1, 512], F32, tag="stA", name="stA")   # mu
                stB = sp.tile([1, 512], F32, tag="stB", name="stB")   # var->rstd
                stC = sp.tile([1, 512], F32, tag="stC", name="stC")   # mu2->mrs
                stI = sp.tile([1, 512], I32, tag="stI", name="stI")
                stD = sp.tile([1, 512], F32, tag="stD", name="stD")
                nc.vector.tensor_copy(stA[:], mu_ps[:])
                nc.vector.tensor_mul(stC[:], stA[:], stA[:])
                nc.vector.scalar_tensor_tensor(stB[:], msq_ps[:], EPS, stC[:],
                                               Alu.add, Alu.subtract)  # var
                yield
                # rsqrt: magic seed  y0 = bits(0x5F3759DF - (bits(v) >> 1))
                nc.vector.tensor_scalar(stI[:], stB[:].bitcast(I32), 1, None,
                                        Alu.arith_shift_right)
                nc.vector.tensor_scalar(stI[:], stI[:], -1, 0x5F3759E0,
                                        Alu.bitwise_xor, Alu.add)
                y0 = stI[:].bitcast(F32)
                # one Newton step: y1 = y0 * (1.5 - 0.5 * v * y0^2), stt-fused
                nc.vector.tensor_mul(stD[:], y0, y0)
                nc.vector.scalar_tensor_tensor(stD[:], stD[:], -0.5, stB[:],
                                               Alu.mult, Alu.mult)
                nc.vector.scalar_tensor_tensor(stB[:], stD[:], 1.5, y0,
                                               Alu.add, Alu.mult)     # rstd
                nc.vector.tensor_mul(stC[:], stA[:], stB[:])          # mu*rstd
                yield
                for cc in range(2):
                    g_row = gt[0:1, ln_id, cc * P:(cc + 1) * P]
                    A_ps = ps.tile([P, 512], F32, tag="mm", name="A_ps")
                    nc.tensor.matmul(A_ps[:], lhsT=_r(g_row), rhs=_r(stB[:]),
                                     start=True, stop=True)
                    B_ps = ps.tile([P, 512], F32, tag="mm", name="B_ps")
                    nc.tensor.matmul(B_ps[:], lhsT=_r(g_row), rhs=_r(stC[:]),
                                     start=True, stop=True)
                    tmp = sp.tile([P, 512], F32, tag="lntmp", name="lntmp")
                    nc.vector.tensor_mul(tmp[:], src[cc][:, sl], A_ps[:])
                    nc.vector.scalar_tensor_tensor(out[cc][:, sl], tmp[:],
                                                   b_of(cc), B_ps[:],
                                                   Alu.add, Alu.subtract)
                    yield

            # ---------------- qkv production ----------------
            def emit_qkv_local_half(l, hf, h, q, k, v):
                tsl = slice(hf * 512, (hf + 1) * 512)
                for mt in range(2):
                    qps = ps.tile([P, 512], F32, tag="mm", name="qps")
                    for kc in range(2):
                        nc.tensor.matmul(qps[:],
                                         lhsT=wq[l][:, kc, mt * P:(mt + 1) * P],
                                         rhs=h[kc][:, tsl],
                                         start=(kc == 0), stop=(kc == 1))
                    nc.vector.tensor_copy(q[mt][:, tsl], qps[:])
                    yield
                    kps = ps.tile([P, 512], F32, tag="mm", name="kps")
                    for kc in range(2):
                        nc.tensor.matmul(kps[:],
                                         lhsT=wkt[l][:, kc, mt * P:(mt + 1) * P],
                                         rhs=h[kc][:, tsl],
                                         start=(kc == 0), stop=(kc == 1))
                    nc.vector.tensor_copy(k[mt][:, tsl], kps[:])
                    yield
                for st in range(hf * 4, hf * 4 + 4):
                    vps = ps.tile([P, C], F32, tag="mm", name="vps")
                    for kc in range(2):
                        nc.tensor.matmul(vps[:],
                                         lhsT=h[kc][:, st * P:(st + 1) * P],
                                         rhs=wv[l][:, kc, :],
                                         start=(kc == 0), stop=(kc == 1))
                    nc.gpsimd.memset(v[st][:, :, 32:33], 1.0)
                    nc.vector.tensor_copy(v[st][:, :, 0:32], vps[:])
                    yield

            def emit_remote(l, h, k, v):
                """AllGather h between the pair; k/v for the remote s-half.
                The DMA/collective part is emitted immediately; the dependent
                matmuls yield for co-emission."""
                b_in = dp.tile([2 * P, TL], BF16, tag="b_in", name="b_in")
                b_out = dp.tile([4 * P, TL], BF16, tag="b_out", name="b_out")
                for cc in range(2):
                    nc.sync.dma_start(b_in[cc * P:(cc + 1) * P, :], h[cc][:])
                if sim:
                    nc.sync.dma_start(b_out[:2 * P, :], b_in[:])
                    nc.sync.dma_start(b_out[2 * P:, :], b_in[:])
                else:
                    nc.gpsimd.collective_compute(
                        "AllGather", Alu.bypass, replica_groups=REPL,
                        ins=[b_in[:].opt()], outs=[b_out[:].opt()])
                hr = wk.tile([P, 2, TL], BF16, tag="hR", bufs=2, name="hR")
                nc.gpsimd.dma_gather(hr[:], b_out[:], remidx_sb[:], 2 * P, 2 * P,
                                     TL)
                yield
                for mt in range(2):
                    for nch in (2, 3):
                        kps = ps.tile([P, 512], F32, tag="mm", name="kps")
                        for kc in range(2):
                            nc.tensor.matmul(
                                kps[:], lhsT=wkt[l][:, kc, mt * P:(mt + 1) * P],
                                rhs=hr[:, kc, (nch - 2) * 512:(nch - 1) * 512],
                                start=(kc == 0), stop=(kc == 1))
                        nc.vector.tensor_copy(k[mt][:, nch * 512:(nch + 1) * 512],
                                              kps[:])
                        yield
                for st in range(8, 16):
                    vps = ps.tile([P, C], F32, tag="mm", name="vps")
                    for kc in range(2):
                        nc.tensor.matmul(vps[:],
                                         lhsT=hr[:, kc, (st - 8) * P:(st - 7) * P],
                                         rhs=wv[l][:, kc, :],
                                         start=(kc == 0), stop=(kc == 1))
                    nc.gpsimd.memset(v[st][:, :, 32:33], 1.0)
                    nc.vector.tensor_copy(v[st][:, :, 0:32], vps[:])
                    yield

            def alloc_attn_tiles():
                q = [wk.tile([P, TL], BF16, tag=f"qT{mt}", bufs=2,
                             name=f"qT{mt}") for mt in range(2)]
                k = [wk.tile([P, T], BF16, tag=f"kT{mt}", bufs=2,
                             name=f"kT{mt}") for mt in range(2)]
                v = [wk.tile([P, H, 33], BF16, tag=f"v{st}", bufs=2,
                             name=f"v{st}") for st in range(NS)]
                h = [wk.tile([P, TL], BF16, tag=f"hT{cc}", bufs=2,
                             name=f"hT{cc}") for cc in range(2)]
                return q, k, v, h

            # generator pump: emit a few chunks of deferred work per st slot.
            # Each generator carries a deadline (scheduling point by which its
            # instructions must have been emitted, because a later batch's
            # matmuls read its outputs and engines execute in emission order).
            pending = []   # list of [deadline, generator]

            def pump(n=1):
                for _ in range(n):
                    if not pending:
                        return
                    _MARKS.append((f"pump-d{pending[0][0]}", nc.next_id()))
                    try:
                        next(pending[0][1])
                    except StopIteration:
                        pending.pop(0)

            def drain_until(point):
                while pending and pending[0][0] <= point:
                    g = pending[0][1]
                    try:
                        while True:
                            next(g)
                    except StopIteration:
                        pending.pop(0)

            def drain():
                while pending:
                    pump()

            # ---------------- attention ----------------
            def attention_window(tt, o_acc, s0, s1, q, k, v, pump_n=4):
                for st in range(s0, s1):
                    S = ps.tile([P, H * P], F32, tag="S", name="S")
                    for j in range(H):
                        mt, jj = divmod(j, 4)
                        nc.tensor.matmul(
                            S[:, j * P:(j + 1) * P],
                            lhsT=k[mt][32 * jj:32 * (jj + 1),
                                       st * P:(st + 1) * P],
                            rhs=q[mt][32 * jj:32 * (jj + 1),
                                      tt * P:(tt + 1) * P],
                            start=True, stop=True, tile_position=(32 * jj, 0))
                    expS = ep.tile([P, H * P], BF16, tag="expT", name="expS")
                    nc.scalar.activation(expS[:], S[:], Act.Exp, scale=SCALE)
                    for j in range(H):
                        nc.tensor.matmul(o_acc[:, j, :],
                                         lhsT=expS[:, j * P:(j + 1) * P],
                                         rhs=v[st][:, j, :],
                                         start=(st == s0), stop=(st == s1 - 1),
                                         skip_group_check=True)
                    pump(pump_n)

            def attention_finish(tt, o_acc, p):
                if p is not None:
                    nc.vector.tensor_add(p[:], p[:], o_acc[:])
                    src = p
                else:
                    src = o_acc
                rec = sp.tile([P, H], F32, tag="rec", name="rec")
                nc.vector.reciprocal(rec[:], src[:, :, 32])
                o_sb = op.tile([P, C], BF16, tag="o_sb", name="o_sb")
                for j in range(H):
                    nc.vector.tensor_single_scalar(o_sb[:, j * HS:(j + 1) * HS],
                                                   src[:, j, 0:32],
                                                   rec[:, j:j + 1], Alu.mult)
                for cc in range(2):
                    nc.sync.dma_start_transpose(
                        oT_sb[:, cc, tt * P:(tt + 1) * P],
                        o_sb[:, cc * P:(cc + 1) * P])

            # ---------------- post-attention (per t-half) ----------------
            def post_half(l, hf, nxt):
                tsl = slice(hf * 512, (hf + 1) * 512)
                for cc in range(2):
                    dpj = ps.tile([P, 512], F32, tag="mm", name="dpj")
                    for kc in range(2):
                        nc.tensor.matmul(dpj[:],
                                         lhsT=wp[l][:, kc, cc * P:(cc + 1) * P],
                                         rhs=oT_sb[:, kc, tsl],
                                         start=(kc == 0), stop=(kc == 1))
                    nc.vector.scalar_tensor_tensor(xT[cc][:, tsl], dpj[:],
                                                   vap(l, 4, cc), xT[cc][:, tsl],
                                                   Alu.add, Alu.add)
                    yield
                yield from emit_ln_half(xT, 2 * l + 1,
                                        lambda cc: vap(l, 3, cc), h2T, hf)
                for ff in range(2):
                    fps = ps.tile([P, 512], F32, tag="mm", name="fps")
                    for kc in range(2):
                        nc.tensor.matmul(fps[:],
                                         lhsT=w1[l][:, kc, ff * P:(ff + 1) * P],
                                         rhs=h2T[kc][:, tsl],
                                         start=(kc == 0), stop=(kc == 1))
                    nc.vector.tensor_scalar(fT[ff][:, tsl], fps[:], vap(l, 5, ff),
                                            0.0, Alu.add, Alu.max)
                    yield
                for cc in range(2):
                    d2 = ps.tile([P, 512], F32, tag="mm", name="d2")
                    for kc in range(2):
                        nc.tensor.matmul(d2[:],
                                         lhsT=w2[l][:, kc, cc * P:(cc + 1) * P],
                                         rhs=fT[kc][:, tsl],
                                         start=(kc == 0), stop=(kc == 1))
                    nc.vector.scalar_tensor_tensor(xT[cc][:, tsl], d2[:],
                                                   vap(l, 6, cc), xT[cc][:, tsl],
                                                   Alu.add, Alu.add)
                    yield
                if l + 1 < L:
                    qn, kn, vn, hn = nxt
                    yield from emit_ln_half(xT, 2 * (l + 1),
                                            lambda cc: vap(l + 1, 1, cc), hn, hf)
                    yield from emit_qkv_local_half(l + 1, hf, hn, qn, kn, vn)
                else:
                    yield from emit_ln_half(xT, 2 * L,
                                            lambda cc: lnf[:, 1, cc:cc + 1],
                                            xf, hf)
                    for cc in range(2):
                        nc.vector.reduce_sum(emb2[:, hf, cc:cc + 1],
                                             xf[cc][:, tsl], axis=X_AXIS)
                    yield

            # ---------------- embedding + layer-0 prologue ----------------
            # bf16 token rows gathered straight into x^T layout (16-bit xbar
            # transpose in the gather DMA); fp32 residual formed by adding the
            # host-pretransposed fp32 positional embeddings.
            with tc.tile_pool(name="embed", bufs=1) as ebp:
                xg = ebp.tile([P, 2, TL], BF16, tag="xg")
                nc.gpsimd.dma_gather(xg[:], tok[:], idx_sb[:], TL, TL, C,
                                     transpose=True)
                pos_sb = ebp.tile([P, 2, TL], F32, tag="pos_sb")
                nc.sync.dma_start(pos_sb[:], posr[:])
                cur = alloc_attn_tiles()
                q0, k0, v0, h0 = cur
                for hf in range(2):
                    tsl = slice(hf * 512, (hf + 1) * 512)
                    for cc in range(2):
                        nc.vector.tensor_add(xT[cc][:, tsl], xg[:, cc, tsl],
                                             pos_sb[:, cc, tsl])
                    for _ in emit_ln_half(xT, 0, lambda cc: vap(0, 1, cc),
                                          h0, hf):
                        pass
                    for _ in emit_qkv_local_half(0, hf, h0, q0, k0, v0):
                        pass
                pending.append([60, emit_remote(0, h0, k0, v0)])

            # ---------------- transformer layers ----------------
            # Each layer processes the B-half t-tiles (4-7) FIRST so post-B
            # (which feeds the next layer's B tiles + the AllGather) runs
            # mid-layer, hidden under the A-tiles' exp stream; post-A runs at
            # the layer end and its serial chain is hidden by the next layer's
            # opening wave of (B-tile x s4-7) scores, which depend only on
            # post-B outputs.  Every tile runs in three s-windows with
            # partial-sum evacuation to SBUF, so 2 PSUM acc slots support a
            # 16-exp runway and all finishes read cheap SBUF partials.
            # Wave points: 0 = needs prior post-B only, 30 = needs post-A of
            # the previous layer (k/q A-half), 60 = needs this layer's remote.
            for l in range(L):
                q, k, v, h = cur
                nxt = alloc_attn_tiles() if l + 1 < L else None
                part = {}
                wincnt = {}
                TB, TA = (4, 5, 6, 7), (0, 1, 2, 3)
                sched = ([(tt, 4, 8, 0) for tt in TB]
                         + [(tt, 0, 4, 30) for tt in TB]
                         + [(tt, 8, 16, 60) for tt in TB]
                         + [(tt, s0, s1, 60) for tt in TA
                            for s0, s1 in ((0, 4), (4, 8), (8, 16))])
                for tt, s0, s1, pt in sched:
                    drain_until(l * 1000 + pt)
                    _MARKS.append((f"L{l}-tt{tt}-s{s0}", nc.next_id()))
                    o_acc = ps.tile([P, H, 33], F32, tag="acc",
                                    padded_shape=[P, H, 64], name="o_acc")
                    attention_window(tt, o_acc, s0, s1, q, k, v)
                    n = wincnt[tt] = wincnt.get(tt, 0) + 1
                    if n == 1:
                        p = wk.tile([P, H, 33], F32, tag=f"part{tt}", bufs=2,
                                    name=f"part{tt}")
                        nc.vector.tensor_copy(p[:], o_acc[:])
                        part[tt] = p
                    elif n == 2:
                        nc.vector.tensor_add(part[tt][:], part[tt][:], o_acc[:])
                    else:
                        attention_finish(tt, o_acc, part.pop(tt))
                        if tt == 7:
                            pending.append([(l + 1) * 1000 + 0,
                                            post_half(l, 1, nxt)])
                        if tt == 3:
                            pending.append([(l + 1) * 1000 + 30,
                                            post_half(l, 0, nxt)])
                            if l + 1 < L:
                                qn, kn, vn, hn = nxt
                                pending.append([(l + 1) * 1000 + 60,
                                                emit_remote(l + 1, hn, kn, vn)])
                cur = nxt
            drain()

            # ---------------- pool + classifier ----------------
            emb = sp.tile([P, 2], F32, tag="emb", bufs=1)
            nc.vector.tensor_add(emb[:], emb2[:, 0, :], emb2[:, 1, :])
            be_in = dp.tile([P, 2], F32, tag="be_in", name="be_in")
            be_out = dp.tile([P, 2], F32, tag="be_out", name="be_out")
            nc.sync.dma_start(be_in[:], emb[:])
            if sim:
                nc.sync.dma_start(be_out[:], be_in[:])
            else:
                nc.gpsimd.collective_compute(
                    "AllReduce", Alu.add, replica_groups=REPL,
                    ins=[be_in[:].opt()], outs=[be_out[:].opt()])
            embr = sp.tile([P, 2], F32, tag="embr", bufs=1)
            nc.sync.dma_start(embr[:], be_out[:])

            h1ps = ps.tile([P, CLS_H // P], F32, tag="mm", name="h1ps")
            for mt in range(CLS_H // P):
                for kc in range(2):
                    nc.tensor.matmul(h1ps[:, mt:mt + 1],
                                     lhsT=wc1[:, kc, mt * P:(mt + 1) * P],
                                     rhs=embr[:, kc:kc + 1],
                                     start=(kc == 0), stop=(kc == 1))
            h1 = sp.tile([P, CLS_H // P], F32, tag="h1", bufs=1)
            nc.vector.tensor_add(h1[:], h1ps[:], bc1[:])
            nc.vector.tensor_scalar_max(h1[:], h1[:], 0.0)
            lps = ps.tile([1, NOUT], F32, tag="mm", name="lps")
            for j in range(CLS_H // P):
                nc.tensor.matmul(lps[:], lhsT=h1[:, j:j + 1], rhs=wc2[:, j, :],
                                 start=(j == 0), stop=(j == CLS_H // P - 1))
            lsb = sp.tile([1, NOUT], F32, tag="lsb", bufs=1)
            nc.vector.tensor_add(lsb[:], lps[:], bc2[:])
            mx = sp.tile([1, 1], F32, tag="mx", bufs=1)
            nc.vector.tensor_reduce(mx[:], lsb[:], axis=X_AXIS, op=Alu.max)
            nmx = sp.tile([1, 1], F32, tag="nmx", bufs=1)
            nc.vector.tensor_scalar_mul(nmx[:], mx[:], -1.0)
            esb = sp.tile([1, NOUT], F32, tag="esb", bufs=1)
            nc.scalar.activation(esb[:], lsb[:], Act.Exp, bias=nmx[:])
            ssum = sp.tile([1, 1], F32, tag="ssum", bufs=1)
            nc.vector.reduce_sum(ssum[:], esb[:], axis=X_AXIS)
            rsum = sp.tile([1, 1], F32, tag="rsum", bufs=1)
            nc.vector.reciprocal(rsum[:], ssum[:])
            probs = sp.tile([1, NOUT], F32, tag="probs", bufs=1)
            nc.vector.tensor_single_scalar(probs[:], esb[:], rsum[:], Alu.mult)
            nc.sync.dma_start(out_d[:], probs[:])

    nc.compile()
    return nc


def _prep_shared(inputs):
    """Host-side weight prepack (identical for all cores)."""
    f = lambda a: np.ascontiguousarray(np.asarray(a, dtype=np.float32))

    def pack_mat(w, dtyp=BF):  # [C_in, M] -> [128, C_in//128, M]
        ci, m = w.shape
        return np.ascontiguousarray(
            w.reshape(ci // P, P, m).transpose(1, 0, 2).astype(dtyp))

    wq3 = np.stack([pack_mat(f(inputs["Wq"][l]).transpose(1, 0, 2).reshape(C, H * HS))
                    for l in range(L)])
    wk3 = np.stack([pack_mat(f(inputs["Wk"][l]).transpose(1, 0, 2).reshape(C, H * HS))
                    for l in range(L)])
    wv3 = np.stack([pack_mat(f(inputs["Wv"][l]).transpose(1, 0, 2).reshape(C, H * HS))
                    for l in range(L)])
    wp3 = np.stack([pack_mat(f(inputs["Wproj"][l])) for l in range(L)])
    w13 = np.stack([pack_mat(f(inputs["W1"][l])) for l in range(L)])
    w23 = np.stack([pack_mat(f(inputs["W2"][l])) for l in range(L)])

    def pack_vec(v):  # [256] -> [128, 2]
        return np.ascontiguousarray(f(v).reshape(2, P).T)

    vecs = np.stack([np.stack([pack_vec(inputs[k][l]) for k in
                               ("ln1_g", "ln1_b", "ln2_g", "ln2_b",
                                "bproj", "b1", "b2")]).transpose(1, 0, 2)
                     for l in range(L)])
    vecs = np.ascontiguousarray(vecs)
    gt_rows = []
    for l in range(L):
        gt_rows.append(f(inputs["ln1_g"][l]))
        gt_rows.append(f(inputs["ln2_g"][l]))
    gt_rows.append(f(inputs["lnf_g"]))
    gt = np.ascontiguousarray(np.stack(gt_rows)[None])   # [1, 2L+1, C]
    lnfv = np.ascontiguousarray(
        np.stack([pack_vec(inputs["lnf_g"]),
                  pack_vec(inputs["lnf_b"])]).transpose(1, 0, 2))
    wc1 = pack_mat(f(inputs["Wc1"]) / T, np.float32)  # fold mean-pool 1/T
    bc1 = np.ascontiguousarray(f(inputs["bc1"]).reshape(CLS_H // P, P).T)
    wc2 = np.ascontiguousarray(f(inputs["Wc2"]).reshape(CLS_H // P, P, NOUT)
                               .transpose(1, 0, 2))
    bc2 = f(inputs["bc2"]).reshape(1, NOUT)
    tokf = np.ascontiguousarray(f(inputs["tok_emb"]).astype(BF))
    posf = f(inputs["pos_emb"])
    return dict(wq=wq3, wk=wk3, wv=wv3, wp=wp3, w1=w13, w2=w23, vecs=vecs,
                gt=gt, lnf=lnfv, wc1=wc1, bc1=bc1, wc2=wc2, bc2=bc2, tok=tokf,
                pos=posf)


def _wrap_idx(ids):
    """int array [n] -> dma_gather wrapped layout [128, n//16] int16."""
    n = ids.shape[0]
    w = ids.reshape(n // 16, 16).T.astype(np.int16)     # [16, n//16]
    return np.ascontiguousarray(np.tile(w, (8, 1)))     # [128, n//16]


def _make_in_maps(inputs):
    shared = _prep_shared(inputs)
    idx = np.asarray(inputs["idx"]).astype(np.int64)
    in_maps = []
    for c in range(N_CORES):
        b, th = c // 2, c % 2
        t0 = th * TL
        idx_loc = idx[b, t0:t0 + TL]
        pos_loc = shared["pos"][t0:t0 + TL]  # [TL, C]
        # pos^T chunks: posr[p, cc, t] = pos[t, cc*128 + p]
        posr_a = np.ascontiguousarray(
            pos_loc.T.reshape(2, P, TL).transpose(1, 0, 2))
        rem = (1 - th) * 2 * P + np.arange(2 * P, dtype=np.int64)
        m = dict(tok=shared["tok"], idxw=_wrap_idx(idx_loc), posr=posr_a,
                 remidx=_wrap_idx(rem),
                 wq=shared["wq"], wk=shared["wk"], wv=shared["wv"],
                 wp=shared["wp"], w1=shared["w1"], w2=shared["w2"],
                 vecs=shared["vecs"], gt=shared["gt"], lnf=shared["lnf"],
                 wc1=shared["wc1"], bc1=shared["bc1"], wc2=shared["wc2"],
                 bc2=shared["bc2"])
        in_maps.append(m)
    return in_maps


def kernel(**inputs) -> np.ndarray:
    if "nc" not in _CACHE:
        _CACHE["nc"] = _build_program()
    nc = _CACHE["nc"]
    in_maps = _make_in_maps(inputs)
    res = bass_utils.run_bass_kernel_spmd(nc, in_maps, core_ids=list(range(N_CORES)))
    out = np.zeros((B, NOUT), np.float32)
    for b in range(B):
        out[b] = res.results[2 * b]["probs"][0]
    return out


# revision 22
# speedup vs baseline: 1.0067x; 1.0067x over previous
"""Trainium2 Bass kernel for nn_EncoderWithClassifier (4-layer encoder + classifier).

Sharding: 8 cores, core c handles (batch b=c//2, sequence half th=c%2, 1024 tokens).
Canonical activation layout: x^T [C=256 (2 chunks of 128 partitions), T_local=1024].

Attention (per 128-token t-tile): scores S [s_tile=128, 8 heads x 128 t] are built
with bf16 K=32 row-packed matmuls, exp'd in one Act op straight to bf16 SBUF, and
o is accumulated UNTRANSPOSED as out[t,d] = expS^T @ V_aug where V_aug carries a
ones column so the softmax denominator falls out of the same matmul (no separate
rowsum matmuls).  o is normalized during PSUM evacuation (per-head [P,1] scalars)
and transposed back to [c,t] with SBUF->SBUF DMA transposes (2-byte xbar path),
keeping PE/DVE free.

All heavy matmuls run 1 cycle/row in the PE: bf16 for attention/proj/FFN weights,
float32r (bitcast) for fp32 LayerNorm statistics.  LN is h = x*A - B with
A = g (x) rstd and B = g (x) (mu*rstd) - b built by outer-product matmuls; rstd
is a bit-hack + Newton rsqrt on the DVE so the Act engine runs EXP ONLY (a single
act-table load for the whole program - table switches would cost 1.3us each).

The Act engine is the roofline (B*H*T^2/8/128 exp elems/core, ~133us/layer); all
other work is hidden under it: post-attention work (proj, LN2, FFN, next-layer
LN1/QKV) is emitted as generators whose chunks are interleaved ("pumped") into
the attention st-loop, and each layer's s-tiles are scheduled local-A-quarter
first / remote-half last so next-layer scores never wait on the (overlapped)
AllGather of h^T (bf16) between the core pair.

PSUM (8 banks): S [128,1024]f32 x2 (4), o_acc [128,8,33->64]f32 x2 (2),
mm [128,512]f32 x2 (2).
"""
import numpy as np
import ml_dtypes

import concourse.bacc as bacc
import concourse.mybir as mybir
import concourse.tile as tile
from concourse import bass_utils, library_config
from concourse.masks import make_identity

V, C, TMAX, H, L = 32000, 256, 2048, 8, 4
HS, FFN = 32, 256
CLS_H, NOUT = 512, 10
B, T = 4, 2048
TL = 1024          # tokens per core
P = 128
NT = TL // P       # 8 local t-tiles
NS = T // P        # 16 s-tiles
EPS = 1e-5
SCALE = C ** (-0.5)
N_CORES = 8
dt = mybir.dt
F32 = dt.float32
F32R = dt.float32r
BF16 = dt.bfloat16
I32 = dt.int32
Alu = mybir.AluOpType
Act = mybir.ActivationFunctionType
X_AXIS = mybir.AxisListType.X
BF = np.dtype(ml_dtypes.bfloat16)

_CACHE = {}
_MARKS = []


def _r(ap):
    """View an fp32 AP as float32r: 1 PE cycle/row (vs 4) at moving dim >=256."""
    return ap.bitcast(F32R)


def _build_program(sim=False):
    nc = bacc.Bacc("TRN2", target_bir_lowering=False, debug=False,
                   num_devices=1 if sim else N_CORES)

    # ---------------- dram I/O ----------------
    tok = nc.dram_tensor("tok", [V, C], BF16, kind="ExternalInput")
    idxw = nc.dram_tensor("idxw", [P, TL // 16], dt.int16, kind="ExternalInput")
    posr = nc.dram_tensor("posr", [P, 2, TL], F32, kind="ExternalInput")
    remidx = nc.dram_tensor("remidx", [P, (2 * P) // 16], dt.int16,
                            kind="ExternalInput")
    wq_d = nc.dram_tensor("wq", [L, P, 2, C], BF16, kind="ExternalInput")
    wk_d = nc.dram_tensor("wk", [L, P, 2, C], BF16, kind="ExternalInput")
    wv_d = nc.dram_tensor("wv", [L, P, 2, C], BF16, kind="ExternalInput")
    wp_d = nc.dram_tensor("wp", [L, P, 2, C], BF16, kind="ExternalInput")
    w1_d = nc.dram_tensor("w1", [L, P, 2, FFN], BF16, kind="ExternalInput")
    w2_d = nc.dram_tensor("w2", [L, P, 2, C], BF16, kind="ExternalInput")
    vecs_d = nc.dram_tensor("vecs", [L, P, 7, 2], F32, kind="ExternalInput")
    # vecs rows: 0 ln1_g, 1 ln1_b, 2 ln2_g, 3 ln2_b, 4 bproj, 5 b1, 6 b2
    gt_d = nc.dram_tensor("gt", [1, 2 * L + 1, C], F32, kind="ExternalInput")
    # gt rows: 2l = ln1_g(l), 2l+1 = ln2_g(l), 2L = lnf_g   (row-vector layout)
    lnf_d = nc.dram_tensor("lnf", [P, 2, 2], F32, kind="ExternalInput")   # g, b
    wc1_d = nc.dram_tensor("wc1", [P, 2, CLS_H], F32, kind="ExternalInput")
    bc1_d = nc.dram_tensor("bc1", [P, CLS_H // P], F32, kind="ExternalInput")
    wc2_d = nc.dram_tensor("wc2", [P, CLS_H // P, NOUT], F32, kind="ExternalInput")
    bc2_d = nc.dram_tensor("bc2", [1, NOUT], F32, kind="ExternalInput")
    out_d = nc.dram_tensor("probs", [1, NOUT], F32, kind="ExternalOutput")

    REPL = [[0, 1], [2, 3], [4, 5], [6, 7]]

    with tile.TileContext(nc) as tc:
        with (
            tc.tile_pool(name="const", bufs=1) as cp,
            tc.tile_pool(name="work", bufs=1) as wk,
            tc.tile_pool(name="exp", bufs=4) as ep,
            tc.tile_pool(name="small", bufs=2) as sp,
            tc.tile_pool(name="osb", bufs=3) as op,
            tc.tile_pool(name="ps", bufs=2, space="PSUM") as ps,
            tc.tile_pool(name="dram", bufs=2, space="DRAM") as dp,
        ):
            nc.gpsimd.load_library(library_config.mlp)

            # ---------------- constants / weights to SBUF ----------------
            inv256 = cp.tile([P, 1], F32, tag="inv256")
            nc.vector.memset(inv256[:], 1.0 / C)

            def load_const(name, dram_ap, shape, dtype=F32):
                t = cp.tile(shape, dtype, tag=name, name=name)
                nc.sync.dma_start(t[:], dram_ap)
                return t

            wq = [load_const(f"wq{l}", wq_d[l], [P, 2, C], BF16) for l in range(L)]
            wkt = [load_const(f"wk{l}", wk_d[l], [P, 2, C], BF16) for l in range(L)]
            wv = [load_const(f"wv{l}", wv_d[l], [P, 2, C], BF16) for l in range(L)]
            wp = [load_const(f"wp{l}", wp_d[l], [P, 2, C], BF16) for l in range(L)]
            w1 = [load_const(f"w1{l}", w1_d[l], [P, 2, FFN], BF16) for l in range(L)]
            w2 = [load_const(f"w2{l}", w2_d[l], [P, 2, C], BF16) for l in range(L)]
            vecs = [load_const(f"vec{l}", vecs_d[l], [P, 7, 2]) for l in range(L)]
            gt = load_const("gt", gt_d[:], [1, 2 * L + 1, C])
            lnf = load_const("lnf", lnf_d[:], [P, 2, 2])
            wc1 = load_const("wc1", wc1_d[:], [P, 2, CLS_H])
            bc1 = load_const("bc1", bc1_d[:], [P, CLS_H // P])
            wc2 = load_const("wc2", wc2_d[:], [P, CLS_H // P, NOUT])
            bc2 = load_const("bc2", bc2_d[:], [1, NOUT])
            idx_sb = load_const("idx_sb", idxw[:], [P, TL // 16], dt.int16)
            remidx_sb = load_const("remidx_sb", remidx[:], [P, (2 * P) // 16],
                                   dt.int16)

            def vap(l, row, cc):
                return vecs[l][:, row, cc:cc + 1]

            # persistent activations
            xT = [wk.tile([P, TL], F32, tag=f"xT{cc}", name=f"xT{cc}")
                  for cc in range(2)]
            oT_sb = wk.tile([P, 2, TL], BF16, tag="oT", name="oT_sb")
            fT = [wk.tile([P, TL], BF16, tag=f"fT{ff}", name=f"fT{ff}")
                  for ff in range(2)]
            h2T = [wk.tile([P, TL], BF16, tag=f"h2T{cc}", name=f"h2T{cc}")
                   for cc in range(2)]
            xf = [wk.tile([P, TL], F32, tag=f"xf{cc}", name=f"xf{cc}")
                  for cc in range(2)]
            emb2 = sp.tile([P, 2, 2], F32, tag="emb2", bufs=1)  # [t-half, cc]

            # ---------------- layernorm (per t-half) ----------------
            def emit_ln_half(src, ln_id, b_of, out, hf):
                """out[cc][:, half] = LN(src)[:, half]; h = x*A - B with
                A = g(x)rstd, B = g(x)(mu*rstd) - b via outer-product matmuls.
                rstd is computed on the DVE (bit-hack + Newton) so Act stays
                exp-only.  Yields between chunks for co-emission."""
                sl = slice(hf * 512, hf * 512 + 512)
                xsq = sp.tile([P, 2, 512], F32, tag="lnsq", name="lnsq")
                for cc in range(2):
                    nc.gpsimd.tensor_mul(xsq[:, cc, :], src[cc][:, sl],
                                         src[cc][:, sl])
                yield
                mu_ps = ps.tile([1, 512], F32, tag="mm", name="mu_ps")
                for kc in range(2):
                    nc.tensor.matmul(mu_ps[:], lhsT=_r(inv256[:]),
                                     rhs=_r(src[kc][:, sl]),
                                     start=(kc == 0), stop=(kc == 1))
                msq_ps = ps.tile([1, 512], F32, tag="mm", name="msq_ps")
                for cc in range(2):
                    nc.tensor.matmul(msq_ps[:], lhsT=_r(inv256[:]),
                                     rhs=_r(xsq[:, cc, :]),
                                     start=(cc == 0), stop=(cc == 1))
                yield
                stA = sp.tile([1, 512], F32, tag="stA", name="stA")   # mu
                stB = sp.tile([1, 512], F32, tag="stB", name="stB")   # var->rstd
                stC = sp.tile([1, 512], F32, tag="stC", name="stC")   # mu2->mrs
                stI = sp.tile([1, 512], I32, tag="stI", name="stI")
                stD = sp.tile([1, 512], F32, tag="stD", name="stD")
                nc.vector.tensor_copy(stA[:], mu_ps[:])
                nc.vector.tensor_mul(stC[:], stA[:], stA[:])
                nc.vector.scalar_tensor_tensor(stB[:], msq_ps[:], EPS, stC[:],
                                               Alu.add, Alu.subtract)  # var
                yield
                # rsqrt: magic seed  y0 = bits(0x5F3759DF - (bits(v) >> 1))
                nc.vector.tensor_scalar(stI[:], stB[:].bitcast(I32), 1, None,
                                        Alu.arith_shift_right)
                nc.vector.tensor_scalar(stI[:], stI[:], -1, 0x5F3759E0,
                                        Alu.bitwise_xor, Alu.add)
                y0 = stI[:].bitcast(F32)
                # one Newton step: y1 = y0 * (1.5 - 0.5 * v * y0^2), stt-fused
                nc.vector.tensor_mul(stD[:], y0, y0)
                nc.vector.scalar_tensor_tensor(stD[:], stD[:], -0.5, stB[:],
                                               Alu.mult, Alu.mult)
                nc.vector.scalar_tensor_tensor(stB[:], stD[:], 1.5, y0,
                                               Alu.add, Alu.mult)     # rstd
                nc.vector.tensor_mul(stC[:], stA[:], stB[:])          # mu*rstd
                # spacing yields: give the DVE chain above time to execute
                # before emitting the dependent PE matmuls below, so they
                # don't head-of-line-block the attention stream in the PE
                # queue.
                yield
                yield
                yield
                for cc in range(2):
                    g_row = gt[0:1, ln_id, cc * P:(cc + 1) * P]
                    A_ps = ps.tile([P, 512], F32, tag="mm", name="A_ps")
                    nc.tensor.matmul(A_ps[:], lhsT=_r(g_row), rhs=_r(stB[:]),
                                     start=True, stop=True)
                    B_ps = ps.tile([P, 512], F32, tag="mm", name="B_ps")
                    nc.tensor.matmul(B_ps[:], lhsT=_r(g_row), rhs=_r(stC[:]),
                                     start=True, stop=True)
                    tmp = sp.tile([P, 512], F32, tag="lntmp", name="lntmp")
                    nc.vector.tensor_mul(tmp[:], src[cc][:, sl], A_ps[:])
                    nc.vector.scalar_tensor_tensor(out[cc][:, sl], tmp[:],
                                                   b_of(cc), B_ps[:],
                                                   Alu.add, Alu.subtract)
                    yield

            # ---------------- qkv production ----------------
            def emit_qkv_local_half(l, hf, h, q, k, v):
                tsl = slice(hf * 512, (hf + 1) * 512)
                for mt in range(2):
                    qps = ps.tile([P, 512], F32, tag="mm", name="qps")
                    for kc in range(2):
                        nc.tensor.matmul(qps[:],
                                         lhsT=wq[l][:, kc, mt * P:(mt + 1) * P],
                                         rhs=h[kc][:, tsl],
                                         start=(kc == 0), stop=(kc == 1))
                    nc.vector.tensor_copy(q[mt][:, tsl], qps[:])
                    yield
                    kps = ps.tile([P, 512], F32, tag="mm", name="kps")
                    for kc in range(2):
                        nc.tensor.matmul(kps[:],
                                         lhsT=wkt[l][:, kc, mt * P:(mt + 1) * P],
                                         rhs=h[kc][:, tsl],
                                         start=(kc == 0), stop=(kc == 1))
                    nc.vector.tensor_copy(k[mt][:, tsl], kps[:])
                    yield
                for st in range(hf * 4, hf * 4 + 4):
                    vps = ps.tile([P, C], F32, tag="mm", name="vps")
                    for kc in range(2):
                        nc.tensor.matmul(vps[:],
                                         lhsT=h[kc][:, st * P:(st + 1) * P],
                                         rhs=wv[l][:, kc, :],
                                         start=(kc == 0), stop=(kc == 1))
                    nc.gpsimd.memset(v[st][:, :, 32:33], 1.0)
                    nc.vector.tensor_copy(v[st][:, :, 0:32], vps[:])
                    yield

            def emit_remote(l, h, k, v):
                """AllGather h between the pair; k/v for the remote s-half.
                The DMA/collective part is emitted immediately; the dependent
                matmuls yield for co-emission."""
                b_in = dp.tile([2 * P, TL], BF16, tag="b_in", name="b_in")
                b_out = dp.tile([4 * P, TL], BF16, tag="b_out", name="b_out")
                for cc in range(2):
                    nc.sync.dma_start(b_in[cc * P:(cc + 1) * P, :], h[cc][:])
                if sim:
                    nc.sync.dma_start(b_out[:2 * P, :], b_in[:])
                    nc.sync.dma_start(b_out[2 * P:, :], b_in[:])
                else:
                    nc.gpsimd.collective_compute(
                        "AllGather", Alu.bypass, replica_groups=REPL,
                        ins=[b_in[:].opt()], outs=[b_out[:].opt()])
                hr = wk.tile([P, 2, TL], BF16, tag="hR", bufs=2, name="hR")
                nc.gpsimd.dma_gather(hr[:], b_out[:], remidx_sb[:], 2 * P, 2 * P,
                                     TL)
                # spacing yields: the gather takes ~10us of DMA; emitting the
                # dependent matmuls right away would block the PE queue.
                for _ in range(8):
                    yield
                for mt in range(2):
                    for nch in (2, 3):
                        kps = ps.tile([P, 512], F32, tag="mm", name="kps")
                        for kc in range(2):
                            nc.tensor.matmul(
                                kps[:], lhsT=wkt[l][:, kc, mt * P:(mt + 1) * P],
                                rhs=hr[:, kc, (nch - 2) * 512:(nch - 1) * 512],
                                start=(kc == 0), stop=(kc == 1))
                        nc.vector.tensor_copy(k[mt][:, nch * 512:(nch + 1) * 512],
                                              kps[:])
                        yield
                for st in range(8, 16):
                    vps = ps.tile([P, C], F32, tag="mm", name="vps")
                    for kc in range(2):
                        nc.tensor.matmul(vps[:],
                                         lhsT=hr[:, kc, (st - 8) * P:(st - 7) * P],
                                         rhs=wv[l][:, kc, :],
                                         start=(kc == 0), stop=(kc == 1))
                    nc.gpsimd.memset(v[st][:, :, 32:33], 1.0)
                    nc.vector.tensor_copy(v[st][:, :, 0:32], vps[:])
                    yield

            def alloc_attn_tiles():
                q = [wk.tile([P, TL], BF16, tag=f"qT{mt}", bufs=2,
                             name=f"qT{mt}") for mt in range(2)]
                k = [wk.tile([P, T], BF16, tag=f"kT{mt}", bufs=2,
                             name=f"kT{mt}") for mt in range(2)]
                v = [wk.tile([P, H, 33], BF16, tag=f"v{st}", bufs=2,
                             name=f"v{st}") for st in range(NS)]
                h = [wk.tile([P, TL], BF16, tag=f"hT{cc}", bufs=2,
                             name=f"hT{cc}") for cc in range(2)]
                return q, k, v, h

            # generator pump: emit a few chunks of deferred work per st slot.
            # Each generator carries a deadline (scheduling point by which its
            # instructions must have been emitted, because a later batch's
            # matmuls read its outputs and engines execute in emission order).
            pending = []   # list of [deadline, generator]

            def pump(n=1):
                for _ in range(n):
                    if not pending:
                        return
                    _MARKS.append((f"pump-d{pending[0][0]}", nc.next_id()))
                    try:
                        next(pending[0][1])
                    except StopIteration:
                        pending.pop(0)

            def drain_until(point):
                while pending and pending[0][0] <= point:
                    g = pending[0][1]
                    try:
                        while True:
                            next(g)
                    except StopIteration:
                        pending.pop(0)

            def drain():
                while pending:
                    pump()

            # ---------------- attention ----------------
            def attention_window(tt, o_acc, s0, s1, q, k, v, pump_n=1):
                for st in range(s0, s1):
                    S = ps.tile([P, H * P], F32, tag="S", name="S")
                    for j in range(H):
                        mt, jj = divmod(j, 4)
                        nc.tensor.matmul(
                            S[:, j * P:(j + 1) * P],
                            lhsT=k[mt][32 * jj:32 * (jj + 1),
                                       st * P:(st + 1) * P],
                            rhs=q[mt][32 * jj:32 * (jj + 1),
                                      tt * P:(tt + 1) * P],
                            start=True, stop=True, tile_position=(32 * jj, 0))
                    expS = ep.tile([P, H * P], BF16, tag="expT", name="expS")
                    nc.scalar.activation(expS[:], S[:], Act.Exp, scale=SCALE)
                    for j in range(H):
                        nc.tensor.matmul(o_acc[:, j, :],
                                         lhsT=expS[:, j * P:(j + 1) * P],
                                         rhs=v[st][:, j, :],
                                         start=(st == s0), stop=(st == s1 - 1),
                                         skip_group_check=True)
                    pump(pump_n)

            def attention_finish(tt, o_acc, p):
                if p is not None:
                    nc.vector.tensor_add(p[:], p[:], o_acc[:])
                    src = p
                else:
                    src = o_acc
                rec = sp.tile([P, H], F32, tag="rec", name="rec")
                nc.vector.reciprocal(rec[:], src[:, :, 32])
                o_sb = op.tile([P, C], BF16, tag="o_sb", name="o_sb")
                for j in range(H):
                    nc.vector.tensor_single_scalar(o_sb[:, j * HS:(j + 1) * HS],
                                                   src[:, j, 0:32],
                                                   rec[:, j:j + 1], Alu.mult)
                for cc in range(2):
                    nc.sync.dma_start_transpose(
                        oT_sb[:, cc, tt * P:(tt + 1) * P],
                        o_sb[:, cc * P:(cc + 1) * P])

            # ---------------- post-attention (per t-half) ----------------
            def post_half(l, hf, nxt):
                tsl = slice(hf * 512, (hf + 1) * 512)
                for cc in range(2):
                    dpj = ps.tile([P, 512], F32, tag="mm", name="dpj")
                    for kc in range(2):
                        nc.tensor.matmul(dpj[:],
                                         lhsT=wp[l][:, kc, cc * P:(cc + 1) * P],
                                         rhs=oT_sb[:, kc, tsl],
                                         start=(kc == 0), stop=(kc == 1))
                    nc.vector.scalar_tensor_tensor(xT[cc][:, tsl], dpj[:],
                                                   vap(l, 4, cc), xT[cc][:, tsl],
                                                   Alu.add, Alu.add)
                    yield
                yield from emit_ln_half(xT, 2 * l + 1,
                                        lambda cc: vap(l, 3, cc), h2T, hf)
                for ff in range(2):
                    fps = ps.tile([P, 512], F32, tag="mm", name="fps")
                    for kc in range(2):
                        nc.tensor.matmul(fps[:],
                                         lhsT=w1[l][:, kc, ff * P:(ff + 1) * P],
                                         rhs=h2T[kc][:, tsl],
                                         start=(kc == 0), stop=(kc == 1))
                    nc.vector.tensor_scalar(fT[ff][:, tsl], fps[:], vap(l, 5, ff),
                                            0.0, Alu.add, Alu.max)
                    yield
                for cc in range(2):
                    d2 = ps.tile([P, 512], F32, tag="mm", name="d2")
                    for kc in range(2):
                        nc.tensor.matmul(d2[:],
                                         lhsT=w2[l][:, kc, cc * P:(cc + 1) * P],
                                         rhs=fT[kc][:, tsl],
                                         start=(kc == 0), stop=(kc == 1))
                    nc.vector.scalar_tensor_tensor(xT[cc][:, tsl], d2[:],
                                                   vap(l, 6, cc), xT[cc][:, tsl],
                                                   Alu.add, Alu.add)
                    yield
                if l + 1 < L:
                    qn, kn, vn, hn = nxt
                    yield from emit_ln_half(xT, 2 * (l + 1),
                                            lambda cc: vap(l + 1, 1, cc), hn, hf)
                    yield from emit_qkv_local_half(l + 1, hf, hn, qn, kn, vn)
                else:
                    yield from emit_ln_half(xT, 2 * L,
                                            lambda cc: lnf[:, 1, cc:cc + 1],
                                            xf, hf)
                    for cc in range(2):
                        nc.vector.reduce_sum(emb2[:, hf, cc:cc + 1],
                                             xf[cc][:, tsl], axis=X_AXIS)
                    yield

            # ---------------- embedding + layer-0 prologue ----------------
            # bf16 token rows gathered straight into x^T layout (16-bit xbar
            # transpose in the gather DMA); fp32 residual formed by adding the
            # host-pretransposed fp32 positional embeddings.
            with tc.tile_pool(name="embed", bufs=1) as ebp:
                xg = ebp.tile([P, 2, TL], BF16, tag="xg")
                nc.gpsimd.dma_gather(xg[:], tok[:], idx_sb[:], TL, TL, C,
                                     transpose=True)
                pos_sb = ebp.tile([P, 2, TL], F32, tag="pos_sb")
                nc.sync.dma_start(pos_sb[:], posr[:])
                cur = alloc_attn_tiles()
                q0, k0, v0, h0 = cur
                for hf in range(2):
                    tsl = slice(hf * 512, (hf + 1) * 512)
                    for cc in range(2):
                        nc.vector.tensor_add(xT[cc][:, tsl], xg[:, cc, tsl],
                                             pos_sb[:, cc, tsl])
                    for _ in emit_ln_half(xT, 0, lambda cc: vap(0, 1, cc),
                                          h0, hf):
                        pass
                    for _ in emit_qkv_local_half(0, hf, h0, q0, k0, v0):
                        pass
                pending.append([60, emit_remote(0, h0, k0, v0)])

            # ---------------- transformer layers ----------------
            # Each layer processes the B-half t-tiles (4-7) FIRST so post-B
            # (which feeds the next layer's B tiles + the AllGather) runs
            # mid-layer, hidden under the A-tiles' exp stream; post-A runs at
            # the layer end and its serial chain is hidden by the next layer's
            # opening wave of (B-tile x s4-7) scores, which depend only on
            # post-B outputs.  Every tile runs in three s-windows with
            # partial-sum evacuation to SBUF, so 2 PSUM acc slots support a
            # 16-exp runway and all finishes read cheap SBUF partials.
            # Wave points: 0 = needs prior post-B only, 30 = needs post-A of
            # the previous layer (k/q A-half), 60 = needs this layer's remote.
            for l in range(L):
                q, k, v, h = cur
                nxt = alloc_attn_tiles() if l + 1 < L else None
                part = {}
                wincnt = {}
                TB, TA = (4, 5, 6, 7), (0, 1, 2, 3)
                sched = ([(tt, 4, 8, 0) for tt in TB]
                         + [(tt, 0, 4, 30) for tt in TB]
                         + [(tt, 8, 16, 60) for tt in TB]
                         + [(tt, s0, s1, 60) for tt in TA
                            for s0, s1 in ((0, 4), (4, 8), (8, 16))])
                for tt, s0, s1, pt in sched:
                    drain_until(l * 1000 + pt)
                    _MARKS.append((f"L{l}-tt{tt}-s{s0}", nc.next_id()))
                    o_acc = ps.tile([P, H, 33], F32, tag="acc",
                                    padded_shape=[P, H, 64], name="o_acc")
                    attention_window(tt, o_acc, s0, s1, q, k, v)
                    n = wincnt[tt] = wincnt.get(tt, 0) + 1
                    if n == 1:
                        p = wk.tile([P, H, 33], F32, tag=f"part{tt}", bufs=2,
                                    name=f"part{tt}")
                        nc.vector.tensor_copy(p[:], o_acc[:])
                        part[tt] = p
                    elif n == 2:
                        nc.vector.tensor_add(part[tt][:], part[tt][:], o_acc[:])
                    else:
                        attention_finish(tt, o_acc, part.pop(tt))
                        if tt == 7:
                            pending.append([(l + 1) * 1000 + 0,
                                            post_half(l, 1, nxt)])
                        if tt == 3:
                            pending.append([(l + 1) * 1000 + 30,
                                            post_half(l, 0, nxt)])
                            if l + 1 < L:
                                qn, kn, vn, hn = nxt
                                pending.append([(l + 1) * 1000 + 60,
                                                emit_remote(l + 1, hn, kn, vn)])
                cur = nxt
            drain()

            # ---------------- pool + classifier ----------------
            emb = sp.tile([P, 2], F32, tag="emb", bufs=1)
            nc.vector.tensor_add(emb[:], emb2[:, 0, :], emb2[:, 1, :])
            be_in = dp.tile([P, 2], F32, tag="be_in", name="be_in")
            be_out = dp.tile([P, 2], F32, tag="be_out", name="be_out")
            nc.sync.dma_start(be_in[:], emb[:])
            if sim:
                nc.sync.dma_start(be_out[:], be_in[:])
            else:
                nc.gpsimd.collective_compute(
                    "AllReduce", Alu.add, replica_groups=REPL,
                    ins=[be_in[:].opt()], outs=[be_out[:].opt()])
            embr = sp.tile([P, 2], F32, tag="embr", bufs=1)
            nc.sync.dma_start(embr[:], be_out[:])

            h1ps = ps.tile([P, CLS_H // P], F32, tag="mm", name="h1ps")
            for mt in range(CLS_H // P):
                for kc in range(2):
                    nc.tensor.matmul(h1ps[:, mt:mt + 1],
                                     lhsT=wc1[:, kc, mt * P:(mt + 1) * P],
                                     rhs=embr[:, kc:kc + 1],
                                     start=(kc == 0), stop=(kc == 1))
            h1 = sp.tile([P, CLS_H // P], F32, tag="h1", bufs=1)
            nc.vector.tensor_add(h1[:], h1ps[:], bc1[:])
            nc.vector.tensor_scalar_max(h1[:], h1[:], 0.0)
            lps = ps.tile([1, NOUT], F32, tag="mm", name="lps")
            for j in range(CLS_H // P):
                nc.tensor.matmul(lps[:], lhsT=h1[:, j:j + 1], rhs=wc2[:, j, :],
                                 start=(j == 0), stop=(j == CLS_H // P - 1))
            lsb = sp.tile([1, NOUT], F32, tag="lsb", bufs=1)
            nc.vector.tensor_add(lsb[:], lps[:], bc2[:])
            mx = sp.tile([1, 1], F32, tag="mx", bufs=1)
            nc.vector.tensor_reduce(mx[:], lsb[:], axis=X_AXIS, op=Alu.max)
            nmx = sp.tile([1, 1], F32, tag="nmx", bufs=1)
            nc.vector.tensor_scalar_mul(nmx[:], mx[:], -1.0)
            esb = sp.tile([1, NOUT], F32, tag="esb", bufs=1)
            nc.scalar.activation(esb[:], lsb[:], Act.Exp, bias=nmx[:])
            ssum = sp.tile([1, 1], F32, tag="ssum", bufs=1)
            nc.vector.reduce_sum(ssum[:], esb[:], axis=X_AXIS)
            rsum = sp.tile([1, 1], F32, tag="rsum", bufs=1)
            nc.vector.reciprocal(rsum[:], ssum[:])
            probs = sp.tile([1, NOUT], F32, tag="probs", bufs=1)
            nc.vector.tensor_single_scalar(probs[:], esb[:], rsum[:], Alu.mult)
            nc.sync.dma_start(out_d[:], probs[:])

    nc.compile()
    return nc


def _prep_shared(inputs):
    """Host-side weight prepack (identical for all cores)."""
    f = lambda a: np.ascontiguousarray(np.asarray(a, dtype=np.float32))

    def pack_mat(w, dtyp=BF):  # [C_in, M] -> [128, C_in//128, M]
        ci, m = w.shape
        return np.ascontiguousarray(
            w.reshape(ci // P, P, m).transpose(1, 0, 2).astype(dtyp))

    wq3 = np.stack([pack_mat(f(inputs["Wq"][l]).transpose(1, 0, 2).reshape(C, H * HS))
                    for l in range(L)])
    wk3 = np.stack([pack_mat(f(inputs["Wk"][l]).transpose(1, 0, 2).reshape(C, H * HS))
                    for l in range(L)])
    wv3 = np.stack([pack_mat(f(inputs["Wv"][l]).transpose(1, 0, 2).reshape(C, H * HS))
                    for l in range(L)])
    wp3 = np.stack([pack_mat(f(inputs["Wproj"][l])) for l in range(L)])
    w13 = np.stack([pack_mat(f(inputs["W1"][l])) for l in range(L)])
    w23 = np.stack([pack_mat(f(inputs["W2"][l])) for l in range(L)])

    def pack_vec(v):  # [256] -> [128, 2]
        return np.ascontiguousarray(f(v).reshape(2, P).T)

    vecs = np.stack([np.stack([pack_vec(inputs[k][l]) for k in
                               ("ln1_g", "ln1_b", "ln2_g", "ln2_b",
                                "bproj", "b1", "b2")]).transpose(1, 0, 2)
                     for l in range(L)])
    vecs = np.ascontiguousarray(vecs)
    gt_rows = []
    for l in range(L):
        gt_rows.append(f(inputs["ln1_g"][l]))
        gt_rows.append(f(inputs["ln2_g"][l]))
    gt_rows.append(f(inputs["lnf_g"]))
    gt = np.ascontiguousarray(np.stack(gt_rows)[None])   # [1, 2L+1, C]
    lnfv = np.ascontiguousarray(
        np.stack([pack_vec(inputs["lnf_g"]),
                  pack_vec(inputs["lnf_b"])]).transpose(1, 0, 2))
    wc1 = pack_mat(f(inputs["Wc1"]) / T, np.float32)  # fold mean-pool 1/T
    bc1 = np.ascontiguousarray(f(inputs["bc1"]).reshape(CLS_H // P, P).T)
    wc2 = np.ascontiguousarray(f(inputs["Wc2"]).reshape(CLS_H // P, P, NOUT)
                               .transpose(1, 0, 2))
    bc2 = f(inputs["bc2"]).reshape(1, NOUT)
    tokf = np.ascontiguousarray(f(inputs["tok_emb"]).astype(BF))
    posf = f(inputs["pos_emb"])
    return dict(wq=wq3, wk=wk3, wv=wv3, wp=wp3, w1=w13, w2=w23, vecs=vecs,
                gt=gt, lnf=lnfv, wc1=wc1, bc1=bc1, wc2=wc2, bc2=bc2, tok=tokf,
                pos=posf)


def _wrap_idx(ids):
    """int array [n] -> dma_gather wrapped layout [128, n//16] int16."""
    n = ids.shape[0]
    w = ids.reshape(n // 16, 16).T.astype(np.int16)     # [16, n//16]
    return np.ascontiguousarray(np.tile(w, (8, 1)))     # [128, n//16]


def _make_in_maps(inputs):
    shared = _prep_shared(inputs)
    idx = np.asarray(inputs["idx"]).astype(np.int64)
    in_maps = []
    for c in range(N_CORES):
        b, th = c // 2, c % 2
        t0 = th * TL
        idx_loc = idx[b, t0:t0 + TL]
        pos_loc = shared["pos"][t0:t0 + TL]  # [TL, C]
        # pos^T chunks: posr[p, cc, t] = pos[t, cc*128 + p]
        posr_a = np.ascontiguousarray(
            pos_loc.T.reshape(2, P, TL).transpose(1, 0, 2))
        rem = (1 - th) * 2 * P + np.arange(2 * P, dtype=np.int64)
        m = dict(tok=shared["tok"], idxw=_wrap_idx(idx_loc), posr=posr_a,
                 remidx=_wrap_idx(rem),
                 wq=shared["wq"], wk=shared["wk"], wv=shared["wv"],
                 wp=shared["wp"], w1=shared["w1"], w2=shared["w2"],
                 vecs=shared["vecs"], gt=shared["gt"], lnf=shared["lnf"],
                 wc1=shared["wc1"], bc1=shared["bc1"], wc2=shared["wc2"],
                 bc2=shared["bc2"])
        in_maps.append(m)
    return in_maps


def kernel(**inputs) -> np.ndarray:
    if "nc" not in _CACHE:
        _CACHE["nc"] = _build_program()
    nc = _CACHE["nc"]
    in_maps = _make_in_maps(inputs)
    res = bass_utils.run_bass_kernel_spmd(nc, in_maps, core_ids=list(range(N_CORES)))
    out = np.zeros((B, NOUT), np.float32)
    for b in range(B):
        out[b] = res.results[2 * b]["probs"][0]
    return out


# revision 25
# speedup vs baseline: 1.0598x; 1.0528x over previous
"""Trainium2 Bass kernel for nn_EncoderWithClassifier (4-layer encoder + classifier).

Sharding: 8 cores, core c handles (batch b=c//2, sequence half th=c%2, 1024 tokens).
Canonical activation layout: x^T [C=256 (2 chunks of 128 partitions), T_local=1024].

Attention (per 128-token t-tile): scores S [s_tile=128, 8 heads x 128 t] are built
with bf16 K=32 row-packed matmuls, exp'd in one Act op straight to bf16 SBUF, and
o is accumulated UNTRANSPOSED as out[t,d] = expS^T @ V_aug where V_aug carries a
ones column so the softmax denominator falls out of the same matmul (no separate
rowsum matmuls).  o is normalized during PSUM evacuation (per-head [P,1] scalars)
and transposed back to [c,t] with SBUF->SBUF DMA transposes (2-byte xbar path),
keeping PE/DVE free.

All heavy matmuls run 1 cycle/row in the PE: bf16 for attention/proj/FFN weights,
float32r (bitcast) for fp32 LayerNorm statistics.  LN is h = x*A - B with
A = g (x) rstd and B = g (x) (mu*rstd) - b built by outer-product matmuls; rstd
is a bit-hack + Newton rsqrt on the DVE so the Act engine runs EXP ONLY (a single
act-table load for the whole program - table switches would cost 1.3us each).

The Act engine is the roofline (B*H*T^2/8/128 exp elems/core, ~133us/layer); all
other work is hidden under it: post-attention work (proj, LN2, FFN, next-layer
LN1/QKV) is emitted as generators whose chunks are interleaved ("pumped") into
the attention st-loop, and each layer's s-tiles are scheduled local-A-quarter
first / remote-half last so next-layer scores never wait on the (overlapped)
AllGather of h^T (bf16) between the core pair.

PSUM (8 banks): S [128,1024]f32 x2 (4), o_acc [128,8,33->64]f32 x2 (2),
mm [128,512]f32 x2 (2).
"""
import numpy as np
import ml_dtypes

import concourse.bacc as bacc
import concourse.mybir as mybir
import concourse.tile as tile
from concourse import bass_utils, library_config
from concourse.masks import make_identity

V, C, TMAX, H, L = 32000, 256, 2048, 8, 4
HS, FFN = 32, 256
CLS_H, NOUT = 512, 10
B, T = 4, 2048
TL = 1024          # tokens per core
P = 128
NT = TL // P       # 8 local t-tiles
NS = T // P        # 16 s-tiles
EPS = 1e-5
SCALE = C ** (-0.5)
N_CORES = 8
dt = mybir.dt
F32 = dt.float32
F32R = dt.float32r
BF16 = dt.bfloat16
I32 = dt.int32
Alu = mybir.AluOpType
Act = mybir.ActivationFunctionType
X_AXIS = mybir.AxisListType.X
BF = np.dtype(ml_dtypes.bfloat16)

_CACHE = {}
_MARKS = []


def _r(ap):
    """View an fp32 AP as float32r: 1 PE cycle/row (vs 4) at moving dim >=256."""
    return ap.bitcast(F32R)


def _build_program(sim=False):
    nc = bacc.Bacc("TRN2", target_bir_lowering=False, debug=False,
                   num_devices=1 if sim else N_CORES)

    # ---------------- dram I/O ----------------
    tok = nc.dram_tensor("tok", [V, C], BF16, kind="ExternalInput")
    idxw = nc.dram_tensor("idxw", [P, TL // 16], dt.int16, kind="ExternalInput")
    posr = nc.dram_tensor("posr", [P, 2, TL], F32, kind="ExternalInput")
    remidx = nc.dram_tensor("remidx", [P, (2 * P) // 16], dt.int16,
                            kind="ExternalInput")
    wq_d = nc.dram_tensor("wq", [L, P, 2, C], BF16, kind="ExternalInput")
    wk_d = nc.dram_tensor("wk", [L, P, 2, C], BF16, kind="ExternalInput")
    wv_d = nc.dram_tensor("wv", [L, P, 2, C], BF16, kind="ExternalInput")
    wp_d = nc.dram_tensor("wp", [L, P, 2, C], BF16, kind="ExternalInput")
    w1_d = nc.dram_tensor("w1", [L, P, 2, FFN], BF16, kind="ExternalInput")
    w2_d = nc.dram_tensor("w2", [L, P, 2, C], BF16, kind="ExternalInput")
    vecs_d = nc.dram_tensor("vecs", [L, P, 7, 2], F32, kind="ExternalInput")
    # vecs rows: 0 ln1_g, 1 ln1_b, 2 ln2_g, 3 ln2_b, 4 bproj, 5 b1, 6 b2
    gt_d = nc.dram_tensor("gt", [1, 2 * L + 1, C], F32, kind="ExternalInput")
    # gt rows: 2l = ln1_g(l), 2l+1 = ln2_g(l), 2L = lnf_g   (row-vector layout)
    lnf_d = nc.dram_tensor("lnf", [P, 2, 2], F32, kind="ExternalInput")   # g, b
    wc1_d = nc.dram_tensor("wc1", [P, 2, CLS_H], F32, kind="ExternalInput")
    bc1_d = nc.dram_tensor("bc1", [P, CLS_H // P], F32, kind="ExternalInput")
    wc2_d = nc.dram_tensor("wc2", [P, CLS_H // P, NOUT], F32, kind="ExternalInput")
    bc2_d = nc.dram_tensor("bc2", [1, NOUT], F32, kind="ExternalInput")
    out_d = nc.dram_tensor("probs", [1, NOUT], F32, kind="ExternalOutput")

    REPL = [[0, 1], [2, 3], [4, 5], [6, 7]]

    with tile.TileContext(nc) as tc:
        with (
            tc.tile_pool(name="const", bufs=1) as cp,
            tc.tile_pool(name="work", bufs=1) as wk,
            tc.tile_pool(name="exp", bufs=4) as ep,
            tc.tile_pool(name="small", bufs=2) as sp,
            tc.tile_pool(name="osb", bufs=3) as op,
            tc.tile_pool(name="ps", bufs=2, space="PSUM") as ps,
            tc.tile_pool(name="dram", bufs=2, space="DRAM") as dp,
        ):
            nc.gpsimd.load_library(library_config.mlp)

            # ---------------- constants / weights to SBUF ----------------
            inv256 = cp.tile([P, 1], F32, tag="inv256")
            nc.vector.memset(inv256[:], 1.0 / C)

            def load_const(name, dram_ap, shape, dtype=F32):
                t = cp.tile(shape, dtype, tag=name, name=name)
                nc.sync.dma_start(t[:], dram_ap)
                return t

            wq = [load_const(f"wq{l}", wq_d[l], [P, 2, C], BF16) for l in range(L)]
            wkt = [load_const(f"wk{l}", wk_d[l], [P, 2, C], BF16) for l in range(L)]
            wv = [load_const(f"wv{l}", wv_d[l], [P, 2, C], BF16) for l in range(L)]
            wp = [load_const(f"wp{l}", wp_d[l], [P, 2, C], BF16) for l in range(L)]
            w1 = [load_const(f"w1{l}", w1_d[l], [P, 2, FFN], BF16) for l in range(L)]
            w2 = [load_const(f"w2{l}", w2_d[l], [P, 2, C], BF16) for l in range(L)]
            vecs = [load_const(f"vec{l}", vecs_d[l], [P, 7, 2]) for l in range(L)]
            gt = load_const("gt", gt_d[:], [1, 2 * L + 1, C])
            lnf = load_const("lnf", lnf_d[:], [P, 2, 2])
            wc1 = load_const("wc1", wc1_d[:], [P, 2, CLS_H])
            bc1 = load_const("bc1", bc1_d[:], [P, CLS_H // P])
            wc2 = load_const("wc2", wc2_d[:], [P, CLS_H // P, NOUT])
            bc2 = load_const("bc2", bc2_d[:], [1, NOUT])
            idx_sb = load_const("idx_sb", idxw[:], [P, TL // 16], dt.int16)
            remidx_sb = load_const("remidx_sb", remidx[:], [P, (2 * P) // 16],
                                   dt.int16)

            def vap(l, row, cc):
                return vecs[l][:, row, cc:cc + 1]

            # persistent activations
            xT = [wk.tile([P, TL], F32, tag=f"xT{cc}", name=f"xT{cc}")
                  for cc in range(2)]
            oT_sb = wk.tile([P, 2, TL], BF16, tag="oT", name="oT_sb")
            fT = [wk.tile([P, TL], BF16, tag=f"fT{ff}", name=f"fT{ff}")
                  for ff in range(2)]
            h2T = [wk.tile([P, TL], BF16, tag=f"h2T{cc}", name=f"h2T{cc}")
                   for cc in range(2)]
            xf = [wk.tile([P, TL], F32, tag=f"xf{cc}", name=f"xf{cc}")
                  for cc in range(2)]
            emb2 = sp.tile([P, 2, 2], F32, tag="emb2", bufs=1)  # [t-half, cc]

            # ---------------- layernorm (per t-half) ----------------
            def emit_ln_half(src, ln_id, b_of, out, hf):
                """out[cc][:, half] = LN(src)[:, half]; h = x*A - B with
                A = g(x)rstd, B = g(x)(mu*rstd) - b via outer-product matmuls.
                rstd is computed on the DVE (bit-hack + Newton) so Act stays
                exp-only.  Yields between chunks for co-emission."""
                sl = slice(hf * 512, hf * 512 + 512)
                xsq = sp.tile([P, 2, 512], F32, tag="lnsq", name="lnsq")
                for cc in range(2):
                    nc.gpsimd.tensor_mul(xsq[:, cc, :], src[cc][:, sl],
                                         src[cc][:, sl])
                yield
                mu_ps = ps.tile([1, 512], F32, tag="mm", name="mu_ps")
                for kc in range(2):
                    nc.tensor.matmul(mu_ps[:], lhsT=_r(inv256[:]),
                                     rhs=_r(src[kc][:, sl]),
                                     start=(kc == 0), stop=(kc == 1))
                msq_ps = ps.tile([1, 512], F32, tag="mm", name="msq_ps")
                for cc in range(2):
                    nc.tensor.matmul(msq_ps[:], lhsT=_r(inv256[:]),
                                     rhs=_r(xsq[:, cc, :]),
                                     start=(cc == 0), stop=(cc == 1))
                yield
                stA = sp.tile([1, 512], F32, tag="stA", name="stA")   # mu
                stB = sp.tile([1, 512], F32, tag="stB", name="stB")   # var->rstd
                stC = sp.tile([1, 512], F32, tag="stC", name="stC")   # mu2->mrs
                stI = sp.tile([1, 512], I32, tag="stI", name="stI")
                stD = sp.tile([1, 512], F32, tag="stD", name="stD")
                nc.vector.tensor_copy(stA[:], mu_ps[:])
                nc.vector.tensor_mul(stC[:], stA[:], stA[:])
                nc.vector.scalar_tensor_tensor(stB[:], msq_ps[:], EPS, stC[:],
                                               Alu.add, Alu.subtract)  # var
                yield
                # rsqrt: magic seed  y0 = bits(0x5F3759DF - (bits(v) >> 1))
                nc.vector.tensor_scalar(stI[:], stB[:].bitcast(I32), 1, None,
                                        Alu.arith_shift_right)
                nc.vector.tensor_scalar(stI[:], stI[:], -1, 0x5F3759E0,
                                        Alu.bitwise_xor, Alu.add)
                y0 = stI[:].bitcast(F32)
                # one Newton step: y1 = y0 * (1.5 - 0.5 * v * y0^2), stt-fused
                nc.vector.tensor_mul(stD[:], y0, y0)
                nc.vector.scalar_tensor_tensor(stD[:], stD[:], -0.5, stB[:],
                                               Alu.mult, Alu.mult)
                nc.vector.scalar_tensor_tensor(stB[:], stD[:], 1.5, y0,
                                               Alu.add, Alu.mult)     # rstd
                nc.vector.tensor_mul(stC[:], stA[:], stB[:])          # mu*rstd
                # spacing yields: give the DVE chain above time to execute
                # before emitting the dependent PE matmuls below, so they
                # don't head-of-line-block the attention stream in the PE
                # queue.
                yield
                yield
                yield
                for cc in range(2):
                    g_row = gt[0:1, ln_id, cc * P:(cc + 1) * P]
                    A_ps = ps.tile([P, 512], F32, tag="mm", name="A_ps")
                    nc.tensor.matmul(A_ps[:], lhsT=_r(g_row), rhs=_r(stB[:]),
                                     start=True, stop=True)
                    B_ps = ps.tile([P, 512], F32, tag="mm", name="B_ps")
                    nc.tensor.matmul(B_ps[:], lhsT=_r(g_row), rhs=_r(stC[:]),
                                     start=True, stop=True)
                    tmp = sp.tile([P, 512], F32, tag="lntmp", name="lntmp")
                    nc.vector.tensor_mul(tmp[:], src[cc][:, sl], A_ps[:])
                    nc.vector.scalar_tensor_tensor(out[cc][:, sl], tmp[:],
                                                   b_of(cc), B_ps[:],
                                                   Alu.add, Alu.subtract)
                    yield

            # ---------------- qkv production ----------------
            def emit_qkv_local_half(l, hf, h, q, k, v):
                tsl = slice(hf * 512, (hf + 1) * 512)
                for mt in range(2):
                    qps = ps.tile([P, 512], F32, tag="mm", name="qps")
                    for kc in range(2):
                        nc.tensor.matmul(qps[:],
                                         lhsT=wq[l][:, kc, mt * P:(mt + 1) * P],
                                         rhs=h[kc][:, tsl],
                                         start=(kc == 0), stop=(kc == 1))
                    nc.vector.tensor_copy(q[mt][:, tsl], qps[:])
                    yield
                    kps = ps.tile([P, 512], F32, tag="mm", name="kps")
                    for kc in range(2):
                        nc.tensor.matmul(kps[:],
                                         lhsT=wkt[l][:, kc, mt * P:(mt + 1) * P],
                                         rhs=h[kc][:, tsl],
                                         start=(kc == 0), stop=(kc == 1))
                    nc.vector.tensor_copy(k[mt][:, tsl], kps[:])
                    yield
                for st in range(hf * 4, hf * 4 + 4):
                    vps = ps.tile([P, C], F32, tag="mm", name="vps")
                    for kc in range(2):
                        nc.tensor.matmul(vps[:],
                                         lhsT=h[kc][:, st * P:(st + 1) * P],
                                         rhs=wv[l][:, kc, :],
                                         start=(kc == 0), stop=(kc == 1))
                    nc.gpsimd.memset(v[st][:, :, 32:33], 1.0)
                    nc.vector.tensor_copy(v[st][:, :, 0:32], vps[:])
                    yield

            def emit_remote_half(l, h, k, v, hf):
                """AllGather ONE t-half of h between the pair; k/v for the
                corresponding remote s-quarter (s 1024+512*hf ..+512).  The
                peer's B-half exists mid-layer, so gathering halves separately
                unblocks half the remote scores a whole post-chain earlier."""
                tsl = slice(hf * 512, (hf + 1) * 512)
                b_in = dp.tile([2 * P, 512], BF16, tag=f"b_in{hf}", name="b_in")
                b_out = dp.tile([4 * P, 512], BF16, tag=f"b_out{hf}",
                                name="b_out")
                for cc in range(2):
                    nc.sync.dma_start(b_in[cc * P:(cc + 1) * P, :],
                                      h[cc][:, tsl])
                if sim:
                    nc.sync.dma_start(b_out[:2 * P, :], b_in[:])
                    nc.sync.dma_start(b_out[2 * P:, :], b_in[:])
                else:
                    nc.gpsimd.collective_compute(
                        "AllGather", Alu.bypass, replica_groups=REPL,
                        ins=[b_in[:].opt()], outs=[b_out[:].opt()])
                hr = wk.tile([P, 2, 512], BF16, tag=f"hR{hf}", bufs=2, name="hR")
                nc.gpsimd.dma_gather(hr[:], b_out[:], remidx_sb[:], 2 * P, 2 * P,
                                     512)
                # spacing yields: the gather takes ~6us of DMA; emitting the
                # dependent matmuls right away would block the PE queue.
                for _ in range(6):
                    yield
                for mt in range(2):
                    kps = ps.tile([P, 512], F32, tag="mm", name="kps")
                    for kc in range(2):
                        nc.tensor.matmul(
                            kps[:], lhsT=wkt[l][:, kc, mt * P:(mt + 1) * P],
                            rhs=hr[:, kc, :],
                            start=(kc == 0), stop=(kc == 1))
                    nc.vector.tensor_copy(
                        k[mt][:, T + (hf - 2) * 512:T + (hf - 1) * 512], kps[:])
                    yield
                for st in range(8 + 4 * hf, 12 + 4 * hf):
                    vps = ps.tile([P, C], F32, tag="mm", name="vps")
                    for kc in range(2):
                        nc.tensor.matmul(
                            vps[:],
                            lhsT=hr[:, kc, (st - 8 - 4 * hf) * P:
                                  (st - 7 - 4 * hf) * P],
                            rhs=wv[l][:, kc, :],
                            start=(kc == 0), stop=(kc == 1))
                    nc.gpsimd.memset(v[st][:, :, 32:33], 1.0)
                    nc.vector.tensor_copy(v[st][:, :, 0:32], vps[:])
                    yield

            def alloc_attn_tiles():
                q = [wk.tile([P, TL], BF16, tag=f"qT{mt}", bufs=2,
                             name=f"qT{mt}") for mt in range(2)]
                k = [wk.tile([P, T], BF16, tag=f"kT{mt}", bufs=2,
                             name=f"kT{mt}") for mt in range(2)]
                v = [wk.tile([P, H, 33], BF16, tag=f"v{st}", bufs=2,
                             name=f"v{st}") for st in range(NS)]
                h = [wk.tile([P, TL], BF16, tag=f"hT{cc}", bufs=2,
                             name=f"hT{cc}") for cc in range(2)]
                return q, k, v, h

            # generator pump: emit a few chunks of deferred work per st slot.
            # Each generator carries a deadline (scheduling point by which its
            # instructions must have been emitted, because a later batch's
            # matmuls read its outputs and engines execute in emission order).
            pending = []   # list of [deadline, generator]

            def pump(n=1):
                for _ in range(n):
                    if not pending:
                        return
                    _MARKS.append((f"pump-d{pending[0][0]}", nc.next_id()))
                    try:
                        next(pending[0][1])
                    except StopIteration:
                        pending.pop(0)

            def drain_until(point):
                while pending and pending[0][0] <= point:
                    g = pending[0][1]
                    try:
                        while True:
                            next(g)
                    except StopIteration:
                        pending.pop(0)

            def drain():
                while pending:
                    pump()

            # ---------------- attention ----------------
            def attention_window(tt, o_acc, s0, s1, q, k, v, pump_n=1):
                for st in range(s0, s1):
                    S = ps.tile([P, H * P], F32, tag="S", name="S")
                    for j in range(H):
                        mt, jj = divmod(j, 4)
                        nc.tensor.matmul(
                            S[:, j * P:(j + 1) * P],
                            lhsT=k[mt][32 * jj:32 * (jj + 1),
                                       st * P:(st + 1) * P],
                            rhs=q[mt][32 * jj:32 * (jj + 1),
                                      tt * P:(tt + 1) * P],
                            start=True, stop=True, tile_position=(32 * jj, 0))
                    expS = ep.tile([P, H * P], BF16, tag="expT", name="expS")
                    nc.scalar.activation(expS[:], S[:], Act.Exp, scale=SCALE)
                    for j in range(H):
                        nc.tensor.matmul(o_acc[:, j, :],
                                         lhsT=expS[:, j * P:(j + 1) * P],
                                         rhs=v[st][:, j, :],
                                         start=(st == s0), stop=(st == s1 - 1),
                                         skip_group_check=True)
                    pump(pump_n)

            def attention_finish(tt, o_acc, p):
                if p is not None:
                    nc.vector.tensor_add(p[:], p[:], o_acc[:])
                    src = p
                else:
                    src = o_acc
                rec = sp.tile([P, H], F32, tag="rec", name="rec")
                nc.vector.reciprocal(rec[:], src[:, :, 32])
                o_sb = op.tile([P, C], BF16, tag="o_sb", name="o_sb")
                for j in range(H):
                    nc.vector.tensor_single_scalar(o_sb[:, j * HS:(j + 1) * HS],
                                                   src[:, j, 0:32],
                                                   rec[:, j:j + 1], Alu.mult)
                for cc in range(2):
                    nc.sync.dma_start_transpose(
                        oT_sb[:, cc, tt * P:(tt + 1) * P],
                        o_sb[:, cc * P:(cc + 1) * P])

            # ---------------- post-attention (per t-half) ----------------
            def post_half(l, hf, nxt):
                tsl = slice(hf * 512, (hf + 1) * 512)
                for cc in range(2):
                    dpj = ps.tile([P, 512], F32, tag="mm", name="dpj")
                    for kc in range(2):
                        nc.tensor.matmul(dpj[:],
                                         lhsT=wp[l][:, kc, cc * P:(cc + 1) * P],
                                         rhs=oT_sb[:, kc, tsl],
                                         start=(kc == 0), stop=(kc == 1))
                    nc.vector.scalar_tensor_tensor(xT[cc][:, tsl], dpj[:],
                                                   vap(l, 4, cc), xT[cc][:, tsl],
                                                   Alu.add, Alu.add)
                    yield
                yield from emit_ln_half(xT, 2 * l + 1,
                                        lambda cc: vap(l, 3, cc), h2T, hf)
                for ff in range(2):
                    fps = ps.tile([P, 512], F32, tag="mm", name="fps")
                    for kc in range(2):
                        nc.tensor.matmul(fps[:],
                                         lhsT=w1[l][:, kc, ff * P:(ff + 1) * P],
                                         rhs=h2T[kc][:, tsl],
                                         start=(kc == 0), stop=(kc == 1))
                    nc.vector.tensor_scalar(fT[ff][:, tsl], fps[:], vap(l, 5, ff),
                                            0.0, Alu.add, Alu.max)
                    yield
                for cc in range(2):
                    d2 = ps.tile([P, 512], F32, tag="mm", name="d2")
                    for kc in range(2):
                        nc.tensor.matmul(d2[:],
                                         lhsT=w2[l][:, kc, cc * P:(cc + 1) * P],
                                         rhs=fT[kc][:, tsl],
                                         start=(kc == 0), stop=(kc == 1))
                    nc.vector.scalar_tensor_tensor(xT[cc][:, tsl], d2[:],
                                                   vap(l, 6, cc), xT[cc][:, tsl],
                                                   Alu.add, Alu.add)
                    yield
                if l + 1 < L:
                    qn, kn, vn, hn = nxt
                    yield from emit_ln_half(xT, 2 * (l + 1),
                                            lambda cc: vap(l + 1, 1, cc), hn, hf)
                    yield from emit_qkv_local_half(l + 1, hf, hn, qn, kn, vn)
                else:
                    yield from emit_ln_half(xT, 2 * L,
                                            lambda cc: lnf[:, 1, cc:cc + 1],
                                            xf, hf)
                    for cc in range(2):
                        nc.vector.reduce_sum(emb2[:, hf, cc:cc + 1],
                                             xf[cc][:, tsl], axis=X_AXIS)
                    yield

            # ---------------- embedding + layer-0 prologue ----------------
            # bf16 token rows gathered straight into x^T layout (16-bit xbar
            # transpose in the gather DMA); fp32 residual formed by adding the
            # host-pretransposed fp32 positional embeddings.
            with tc.tile_pool(name="embed", bufs=1) as ebp:
                xg = ebp.tile([P, 2, TL], BF16, tag="xg")
                nc.gpsimd.dma_gather(xg[:], tok[:], idx_sb[:], TL, TL, C,
                                     transpose=True)
                pos_sb = ebp.tile([P, 2, TL], F32, tag="pos_sb")
                nc.sync.dma_start(pos_sb[:], posr[:])
                cur = alloc_attn_tiles()
                q0, k0, v0, h0 = cur
                for hf in range(2):
                    tsl = slice(hf * 512, (hf + 1) * 512)
                    for cc in range(2):
                        nc.vector.tensor_add(xT[cc][:, tsl], xg[:, cc, tsl],
                                             pos_sb[:, cc, tsl])
                    for _ in emit_ln_half(xT, 0, lambda cc: vap(0, 1, cc),
                                          h0, hf):
                        pass
                    for _ in emit_qkv_local_half(0, hf, h0, q0, k0, v0):
                        pass
                pending.append([10, emit_remote_half(0, h0, k0, v0, 1)])
                pending.append([60, emit_remote_half(0, h0, k0, v0, 0)])

            # ---------------- transformer layers ----------------
            # Each layer processes the B-half t-tiles (4-7) FIRST so post-B
            # (which feeds the next layer's B tiles + the AllGather) runs
            # mid-layer, hidden under the A-tiles' exp stream; post-A runs at
            # the layer end and its serial chain is hidden by the next layer's
            # opening wave of (B-tile x s4-7) scores, which depend only on
            # post-B outputs.  Every tile runs in three s-windows with
            # partial-sum evacuation to SBUF, so 2 PSUM acc slots support a
            # 16-exp runway and all finishes read cheap SBUF partials.
            # Wave points: 0 = needs prior post-B only, 30 = needs post-A of
            # the previous layer (k/q A-half), 60 = needs this layer's remote.
            for l in range(L):
                q, k, v, h = cur
                nxt = alloc_attn_tiles() if l + 1 < L else None
                part = {}
                wincnt = {}
                TB, TA = (4, 5, 6, 7), (0, 1, 2, 3)
                sched = ([(tt, 4, 8, 0) for tt in TB]
                         + [(tt, 12, 16, 10) for tt in TB]
                         + [(tt, 0, 4, 30) for tt in TB]
                         + [(tt, 8, 12, 60) for tt in TB]
                         + [(tt, s0, s1, 60) for tt in TA
                            for s0, s1 in ((0, 4), (4, 8), (8, 16))])
                nwin = {tt: (4 if tt in TB else 3) for tt in range(NT)}
                for tt, s0, s1, pt in sched:
                    drain_until(l * 1000 + pt)
                    _MARKS.append((f"L{l}-tt{tt}-s{s0}", nc.next_id()))
                    o_acc = ps.tile([P, H, 33], F32, tag="acc",
                                    padded_shape=[P, H, 64], name="o_acc")
                    attention_window(tt, o_acc, s0, s1, q, k, v)
                    n = wincnt[tt] = wincnt.get(tt, 0) + 1
                    if n == 1:
                        p = wk.tile([P, H, 33], F32, tag=f"part{tt}", bufs=2,
                                    name=f"part{tt}")
                        nc.vector.tensor_copy(p[:], o_acc[:])
                        part[tt] = p
                    elif n < nwin[tt]:
                        nc.vector.tensor_add(part[tt][:], part[tt][:], o_acc[:])
                    else:
                        attention_finish(tt, o_acc, part.pop(tt))
                        if tt == 7:
                            pending.append([(l + 1) * 1000 + 0,
                                            post_half(l, 1, nxt)])
                            if l + 1 < L:
                                qn, kn, vn, hn = nxt
                                pending.append(
                                    [(l + 1) * 1000 + 10,
                                     emit_remote_half(l + 1, hn, kn, vn, 1)])
                        if tt == 3:
                            pending.append([(l + 1) * 1000 + 30,
                                            post_half(l, 0, nxt)])
                            if l + 1 < L:
                                qn, kn, vn, hn = nxt
                                pending.append(
                                    [(l + 1) * 1000 + 60,
                                     emit_remote_half(l + 1, hn, kn, vn, 0)])
                cur = nxt
            drain()

            # ---------------- pool + classifier ----------------
            emb = sp.tile([P, 2], F32, tag="emb", bufs=1)
            nc.vector.tensor_add(emb[:], emb2[:, 0, :], emb2[:, 1, :])
            be_in = dp.tile([P, 2], F32, tag="be_in", name="be_in")
            be_out = dp.tile([P, 2], F32, tag="be_out", name="be_out")
            nc.sync.dma_start(be_in[:], emb[:])
            if sim:
                nc.sync.dma_start(be_out[:], be_in[:])
            else:
                nc.gpsimd.collective_compute(
                    "AllReduce", Alu.add, replica_groups=REPL,
                    ins=[be_in[:].opt()], outs=[be_out[:].opt()])
            embr = sp.tile([P, 2], F32, tag="embr", bufs=1)
            nc.sync.dma_start(embr[:], be_out[:])

            h1ps = ps.tile([P, CLS_H // P], F32, tag="mm", name="h1ps")
            for mt in range(CLS_H // P):
                for kc in range(2):
                    nc.tensor.matmul(h1ps[:, mt:mt + 1],
                                     lhsT=wc1[:, kc, mt * P:(mt + 1) * P],
                                     rhs=embr[:, kc:kc + 1],
                                     start=(kc == 0), stop=(kc == 1))
            h1 = sp.tile([P, CLS_H // P], F32, tag="h1", bufs=1)
            nc.vector.tensor_add(h1[:], h1ps[:], bc1[:])
            nc.vector.tensor_scalar_max(h1[:], h1[:], 0.0)
            lps = ps.tile([1, NOUT], F32, tag="mm", name="lps")
            for j in range(CLS_H // P):
                nc.tensor.matmul(lps[:], lhsT=h1[:, j:j + 1], rhs=wc2[:, j, :],
                                 start=(j == 0), stop=(j == CLS_H // P - 1))
            lsb = sp.tile([1, NOUT], F32, tag="lsb", bufs=1)
            nc.vector.tensor_add(lsb[:], lps[:], bc2[:])
            mx = sp.tile([1, 1], F32, tag="mx", bufs=1)
            nc.vector.tensor_reduce(mx[:], lsb[:], axis=X_AXIS, op=Alu.max)
            nmx = sp.tile([1, 1], F32, tag="nmx", bufs=1)
            nc.vector.tensor_scalar_mul(nmx[:], mx[:], -1.0)
            esb = sp.tile([1, NOUT], F32, tag="esb", bufs=1)
            nc.scalar.activation(esb[:], lsb[:], Act.Exp, bias=nmx[:])
            ssum = sp.tile([1, 1], F32, tag="ssum", bufs=1)
            nc.vector.reduce_sum(ssum[:], esb[:], axis=X_AXIS)
            rsum = sp.tile([1, 1], F32, tag="rsum", bufs=1)
            nc.vector.reciprocal(rsum[:], ssum[:])
            probs = sp.tile([1, NOUT], F32, tag="probs", bufs=1)
            nc.vector.tensor_single_scalar(probs[:], esb[:], rsum[:], Alu.mult)
            nc.sync.dma_start(out_d[:], probs[:])

    nc.compile()
    return nc


def _prep_shared(inputs):
    """Host-side weight prepack (identical for all cores)."""
    f = lambda a: np.ascontiguousarray(np.asarray(a, dtype=np.float32))

    def pack_mat(w, dtyp=BF):  # [C_in, M] -> [128, C_in//128, M]
        ci, m = w.shape
        return np.ascontiguousarray(
            w.reshape(ci // P, P, m).transpose(1, 0, 2).astype(dtyp))

    wq3 = np.stack([pack_mat(f(inputs["Wq"][l]).transpose(1, 0, 2).reshape(C, H * HS))
                    for l in range(L)])
    wk3 = np.stack([pack_mat(f(inputs["Wk"][l]).transpose(1, 0, 2).reshape(C, H * HS))
                    for l in range(L)])
    wv3 = np.stack([pack_mat(f(inputs["Wv"][l]).transpose(1, 0, 2).reshape(C, H * HS))
                    for l in range(L)])
    wp3 = np.stack([pack_mat(f(inputs["Wproj"][l])) for l in range(L)])
    w13 = np.stack([pack_mat(f(inputs["W1"][l])) for l in range(L)])
    w23 = np.stack([pack_mat(f(inputs["W2"][l])) for l in range(L)])

    def pack_vec(v):  # [256] -> [128, 2]
        return np.ascontiguousarray(f(v).reshape(2, P).T)

    vecs = np.stack([np.stack([pack_vec(inputs[k][l]) for k in
                               ("ln1_g", "ln1_b", "ln2_g", "ln2_b",
                                "bproj", "b1", "b2")]).transpose(1, 0, 2)
                     for l in range(L)])
    vecs = np.ascontiguousarray(vecs)
    gt_rows = []
    for l in range(L):
        gt_rows.append(f(inputs["ln1_g"][l]))
        gt_rows.append(f(inputs["ln2_g"][l]))
    gt_rows.append(f(inputs["lnf_g"]))
    gt = np.ascontiguousarray(np.stack(gt_rows)[None])   # [1, 2L+1, C]
    lnfv = np.ascontiguousarray(
        np.stack([pack_vec(inputs["lnf_g"]),
                  pack_vec(inputs["lnf_b"])]).transpose(1, 0, 2))
    wc1 = pack_mat(f(inputs["Wc1"]) / T, np.float32)  # fold mean-pool 1/T
    bc1 = np.ascontiguousarray(f(inputs["bc1"]).reshape(CLS_H // P, P).T)
    wc2 = np.ascontiguousarray(f(inputs["Wc2"]).reshape(CLS_H // P, P, NOUT)
                               .transpose(1, 0, 2))
    bc2 = f(inputs["bc2"]).reshape(1, NOUT)
    tokf = np.ascontiguousarray(f(inputs["tok_emb"]).astype(BF))
    posf = f(inputs["pos_emb"])
    return dict(wq=wq3, wk=wk3, wv=wv3, wp=wp3, w1=w13, w2=w23, vecs=vecs,
                gt=gt, lnf=lnfv, wc1=wc1, bc1=bc1, wc2=wc2, bc2=bc2, tok=tokf,
                pos=posf)


def _wrap_idx(ids):
    """int array [n] -> dma_gather wrapped layout [128, n//16] int16."""
    n = ids.shape[0]
    w = ids.reshape(n // 16, 16).T.astype(np.int16)     # [16, n//16]
    return np.ascontiguousarray(np.tile(w, (8, 1)))     # [128, n//16]


def _make_in_maps(inputs):
    shared = _prep_shared(inputs)
    idx = np.asarray(inputs["idx"]).astype(np.int64)
    in_maps = []
    for c in range(N_CORES):
        b, th = c // 2, c % 2
        t0 = th * TL
        idx_loc = idx[b, t0:t0 + TL]
        pos_loc = shared["pos"][t0:t0 + TL]  # [TL, C]
        # pos^T chunks: posr[p, cc, t] = pos[t, cc*128 + p]
        posr_a = np.ascontiguousarray(
            pos_loc.T.reshape(2, P, TL).transpose(1, 0, 2))
        rem = (1 - th) * 2 * P + np.arange(2 * P, dtype=np.int64)
        m = dict(tok=shared["tok"], idxw=_wrap_idx(idx_loc), posr=posr_a,
                 remidx=_wrap_idx(rem),
                 wq=shared["wq"], wk=shared["wk"], wv=shared["wv"],
                 wp=shared["wp"], w1=shared["w1"], w2=shared["w2"],
                 vecs=shared["vecs"], gt=shared["gt"], lnf=shared["lnf"],
                 wc1=shared["wc1"], bc1=shared["bc1"], wc2=shared["wc2"],
                 bc2=shared["bc2"])
        in_maps.append(m)
    return in_maps


def kernel(**inputs) -> np.ndarray:
    if "nc" not in _CACHE:
        _CACHE["nc"] = _build_program()
    nc = _CACHE["nc"]
    in_maps = _make_in_maps(inputs)
    res = bass_utils.run_bass_kernel_spmd(nc, in_maps, core_ids=list(range(N_CORES)))
    out = np.zeros((B, NOUT), np.float32)
    for b in range(B):
        out[b] = res.results[2 * b]["probs"][0]
    return out


# revision 31
# speedup vs baseline: 1.1045x; 1.0422x over previous
"""Trainium2 Bass kernel for nn_EncoderWithClassifier (4-layer encoder + classifier).

Sharding: 8 cores, core c handles (batch b=c//2, sequence half th=c%2, 1024 tokens).
Canonical activation layout: x^T [C=256 (2 chunks of 128 partitions), T_local=1024].

Attention (per 128-token t-tile): scores S [s_tile=128, 8 heads x 128 t] are built
with bf16 K=32 row-packed matmuls, exp'd in one Act op straight to bf16 SBUF, and
o is accumulated UNTRANSPOSED as out[t,d] = expS^T @ V_aug where V_aug carries a
ones column so the softmax denominator falls out of the same matmul (no separate
rowsum matmuls).  o is normalized during PSUM evacuation (per-head [P,1] scalars)
and transposed back to [c,t] with SBUF->SBUF DMA transposes (2-byte xbar path),
keeping PE/DVE free.

All heavy matmuls run 1 cycle/row in the PE: bf16 for attention/proj/FFN weights,
float32r (bitcast) for fp32 LayerNorm statistics.  LN is h = x*A - B with
A = g (x) rstd and B = g (x) (mu*rstd) - b built by outer-product matmuls; rstd
is a bit-hack + Newton rsqrt on the DVE so the Act engine runs EXP ONLY (a single
act-table load for the whole program - table switches would cost 1.3us each).

The Act engine is the roofline (B*H*T^2/8/128 exp elems/core, ~133us/layer); all
other work is hidden under it: post-attention work (proj, LN2, FFN, next-layer
LN1/QKV) is emitted as generators whose chunks are interleaved ("pumped") into
the attention st-loop, and each layer's s-tiles are scheduled local-A-quarter
first / remote-half last so next-layer scores never wait on the (overlapped)
AllGather of h^T (bf16) between the core pair.

PSUM (8 banks): S [128,1024]f32 x2 (4), o_acc [128,8,33->64]f32 x2 (2),
mm [128,512]f32 x2 (2).
"""
import numpy as np
import ml_dtypes

import concourse.bacc as bacc
import concourse.mybir as mybir
import concourse.tile as tile
from concourse import bass_utils, library_config
from concourse.masks import make_identity

V, C, TMAX, H, L = 32000, 256, 2048, 8, 4
HS, FFN = 32, 256
CLS_H, NOUT = 512, 10
B, T = 4, 2048
TL = 1024          # tokens per core
P = 128
NT = TL // P       # 8 local t-tiles
NS = T // P        # 16 s-tiles
EPS = 1e-5
SCALE = C ** (-0.5)
N_CORES = 8
dt = mybir.dt
F32 = dt.float32
F32R = dt.float32r
BF16 = dt.bfloat16
I32 = dt.int32
Alu = mybir.AluOpType
Act = mybir.ActivationFunctionType
X_AXIS = mybir.AxisListType.X
BF = np.dtype(ml_dtypes.bfloat16)

_CACHE = {}
_MARKS = []


def _r(ap):
    """View an fp32 AP as float32r: 1 PE cycle/row (vs 4) at moving dim >=256."""
    return ap.bitcast(F32R)


def _build_program(sim=False):
    nc = bacc.Bacc("TRN2", target_bir_lowering=False, debug=False,
                   num_devices=1 if sim else N_CORES)

    # ---------------- dram I/O ----------------
    tok = nc.dram_tensor("tok", [V, C], BF16, kind="ExternalInput")
    idxw = nc.dram_tensor("idxw", [P, TL // 16], dt.int16, kind="ExternalInput")
    posr = nc.dram_tensor("posr", [P, 2, TL], F32, kind="ExternalInput")
    remidx = nc.dram_tensor("remidx", [P, (2 * P) // 16], dt.int16,
                            kind="ExternalInput")
    # all bf16 weights in one tensor (one DMA): [P, kind(q,k,v,p,1,2), L, 2, C]
    wsb_d = nc.dram_tensor("wsb", [P, 6, L, 2, C], BF16, kind="ExternalInput")
    # vecs rows: 0 ln1_g, 1 ln1_b, 2 ln2_g, 3 ln2_b, 4 bproj, 5 b1, 6 b2
    vecs_d = nc.dram_tensor("vecs", [P, L, 7, 2], F32, kind="ExternalInput")
    # classifier/lnf consts: wc1 flat [2*512] | wc2 flat [40] | bc1 [4] | lnf [4]
    csb_d = nc.dram_tensor("csb", [P, 2 * CLS_H + 4 * NOUT + 8], F32,
                           kind="ExternalInput")
    # row-vector consts: g rows (2l=ln1_g, 2l+1=ln2_g, 2L=lnf_g) flat | bc2
    gtc_d = nc.dram_tensor("gtc", [1, (2 * L + 1) * C + NOUT], F32,
                           kind="ExternalInput")
    out_d = nc.dram_tensor("probs", [1, NOUT], F32, kind="ExternalOutput")

    REPL = [[0, 1], [2, 3], [4, 5], [6, 7]]

    with tile.TileContext(nc) as tc:
        with (
            tc.tile_pool(name="const", bufs=1) as cp,
            tc.tile_pool(name="work", bufs=1) as wk,
            tc.tile_pool(name="exp", bufs=4) as ep,
            tc.tile_pool(name="small", bufs=2) as sp,
            tc.tile_pool(name="osb", bufs=3) as op,
            tc.tile_pool(name="ps", bufs=2, space="PSUM") as ps,
            tc.tile_pool(name="dram", bufs=2, space="DRAM") as dp,
        ):
            nc.gpsimd.load_library(library_config.mlp)

            # ---------------- constants / weights to SBUF ----------------
            inv256 = cp.tile([P, 1], F32, tag="inv256")
            nc.vector.memset(inv256[:], 1.0 / C)

            def load_const(name, dram_ap, shape, dtype=F32):
                t = cp.tile(shape, dtype, tag=name, name=name)
                nc.sync.dma_start(t[:], dram_ap)
                return t

            wsb = load_const("wsb", wsb_d[:], [P, 6, L, 2, C], BF16)
            wq = [wsb[:, 0, l] for l in range(L)]
            wkt = [wsb[:, 1, l] for l in range(L)]
            wv = [wsb[:, 2, l] for l in range(L)]
            wp = [wsb[:, 3, l] for l in range(L)]
            w1 = [wsb[:, 4, l] for l in range(L)]
            w2 = [wsb[:, 5, l] for l in range(L)]
            vecs = load_const("vecs", vecs_d[:], [P, L, 7, 2])
            csb = load_const("csb", csb_d[:], [P, 2 * CLS_H + 4 * NOUT + 8])
            gtc = load_const("gtc", gtc_d[:], [1, (2 * L + 1) * C + NOUT])
            idx_sb = load_const("idx_sb", idxw[:], [P, TL // 16], dt.int16)
            remidx_sb = load_const("remidx_sb", remidx[:], [P, (2 * P) // 16],
                                   dt.int16)

            def vap(l, row, cc):
                return vecs[:, l, row, cc:cc + 1]

            def g_row_ap(ln_id, cc):
                return gtc[0:1, ln_id * C + cc * P:ln_id * C + cc * P + P]

            def wc1_ap(kc, mt):
                return csb[:, kc * CLS_H + mt * P:kc * CLS_H + (mt + 1) * P]

            def wc2_ap(j):
                return csb[:, 2 * CLS_H + j * NOUT:2 * CLS_H + (j + 1) * NOUT]

            bc1_ap = csb[:, 2 * CLS_H + 4 * NOUT:2 * CLS_H + 4 * NOUT + 4]

            def lnf_b_ap(cc):
                o = 2 * CLS_H + 4 * NOUT + 6 + cc   # lnf block row 1 (beta)
                return csb[:, o:o + 1]

            bc2_ap = gtc[0:1, (2 * L + 1) * C:(2 * L + 1) * C + NOUT]

            # persistent activations
            xT = [wk.tile([P, TL], F32, tag=f"xT{cc}", name=f"xT{cc}")
                  for cc in range(2)]
            oT_sb = wk.tile([P, 2, TL], BF16, tag="oT", name="oT_sb")
            fT = [wk.tile([P, TL], BF16, tag=f"fT{ff}", name=f"fT{ff}")
                  for ff in range(2)]
            h2T = [wk.tile([P, TL], BF16, tag=f"h2T{cc}", name=f"h2T{cc}")
                   for cc in range(2)]
            xf = [wk.tile([P, TL], F32, tag=f"xf{cc}", name=f"xf{cc}")
                  for cc in range(2)]
            emb2 = sp.tile([P, 2, 2], F32, tag="emb2", bufs=1)  # [t-half, cc]

            # ---------------- layernorm (per t-half) ----------------
            def emit_ln_half(src, ln_id, b_of, out, hf):
                """out[cc][:, half] = LN(src)[:, half]; h = x*A - B with
                A = g(x)rstd, B = g(x)(mu*rstd) - b via outer-product matmuls.
                rstd is computed on the DVE (bit-hack + Newton) so Act stays
                exp-only.  Yields between chunks for co-emission."""
                sl = slice(hf * 512, hf * 512 + 512)
                xsq = sp.tile([P, 2, 512], F32, tag="lnsq", name="lnsq")
                for cc in range(2):
                    nc.gpsimd.tensor_mul(xsq[:, cc, :], src[cc][:, sl],
                                         src[cc][:, sl])
                yield
                mu_ps = ps.tile([1, 512], F32, tag="mm", name="mu_ps")
                for kc in range(2):
                    nc.tensor.matmul(mu_ps[:], lhsT=_r(inv256[:]),
                                     rhs=_r(src[kc][:, sl]),
                                     start=(kc == 0), stop=(kc == 1))
                msq_ps = ps.tile([1, 512], F32, tag="mm", name="msq_ps")
                for cc in range(2):
                    nc.tensor.matmul(msq_ps[:], lhsT=_r(inv256[:]),
                                     rhs=_r(xsq[:, cc, :]),
                                     start=(cc == 0), stop=(cc == 1))
                yield
                stA = sp.tile([1, 512], F32, tag="stA", name="stA")   # mu
                stB = sp.tile([1, 512], F32, tag="stB", name="stB")   # var->rstd
                stC = sp.tile([1, 512], F32, tag="stC", name="stC")   # mu2->mrs
                stI = sp.tile([1, 512], I32, tag="stI", name="stI")
                stD = sp.tile([1, 512], F32, tag="stD", name="stD")
                nc.vector.tensor_copy(stA[:], mu_ps[:])
                nc.vector.tensor_mul(stC[:], stA[:], stA[:])
                nc.vector.scalar_tensor_tensor(stB[:], msq_ps[:], EPS, stC[:],
                                               Alu.add, Alu.subtract)  # var
                yield
                # rsqrt: magic seed  y0 = bits(0x5F3759DF - (bits(v) >> 1))
                nc.vector.tensor_scalar(stI[:], stB[:].bitcast(I32), 1, None,
                                        Alu.arith_shift_right)
                nc.vector.tensor_scalar(stI[:], stI[:], -1, 0x5F3759E0,
                                        Alu.bitwise_xor, Alu.add)
                y0 = stI[:].bitcast(F32)
                # one Newton step: y1 = y0 * (1.5 - 0.5 * v * y0^2), stt-fused
                nc.vector.tensor_mul(stD[:], y0, y0)
                nc.vector.scalar_tensor_tensor(stD[:], stD[:], -0.5, stB[:],
                                               Alu.mult, Alu.mult)
                nc.vector.scalar_tensor_tensor(stB[:], stD[:], 1.5, y0,
                                               Alu.add, Alu.mult)     # rstd
                nc.vector.tensor_mul(stC[:], stA[:], stB[:])          # mu*rstd
                # spacing yields: give the DVE chain above time to execute
                # before emitting the dependent PE matmuls below, so they
                # don't head-of-line-block the attention stream in the PE
                # queue.
                yield
                yield
                yield
                for cc in range(2):
                    g_row = g_row_ap(ln_id, cc)
                    A_ps = ps.tile([P, 512], F32, tag="mm", name="A_ps")
                    nc.tensor.matmul(A_ps[:], lhsT=_r(g_row), rhs=_r(stB[:]),
                                     start=True, stop=True)
                    B_ps = ps.tile([P, 512], F32, tag="mm", name="B_ps")
                    nc.tensor.matmul(B_ps[:], lhsT=_r(g_row), rhs=_r(stC[:]),
                                     start=True, stop=True)
                    tmp = sp.tile([P, 512], F32, tag="lntmp", name="lntmp")
                    nc.vector.tensor_mul(tmp[:], src[cc][:, sl], A_ps[:])
                    nc.vector.scalar_tensor_tensor(out[cc][:, sl], tmp[:],
                                                   b_of(cc), B_ps[:],
                                                   Alu.add, Alu.subtract)
                    yield

            # ---------------- qkv production ----------------
            def emit_qkv_local_half(l, hf, h, q, k, v):
                tsl = slice(hf * 512, (hf + 1) * 512)
                for mt in range(2):
                    qps = ps.tile([P, 512], F32, tag="mm", name="qps")
                    for kc in range(2):
                        nc.tensor.matmul(qps[:],
                                         lhsT=wq[l][:, kc, mt * P:(mt + 1) * P],
                                         rhs=h[kc][:, tsl],
                                         start=(kc == 0), stop=(kc == 1))
                    nc.vector.tensor_copy(q[mt][:, tsl], qps[:])
                    yield
                    kps = ps.tile([P, 512], F32, tag="mm", name="kps")
                    for kc in range(2):
                        nc.tensor.matmul(kps[:],
                                         lhsT=wkt[l][:, kc, mt * P:(mt + 1) * P],
                                         rhs=h[kc][:, tsl],
                                         start=(kc == 0), stop=(kc == 1))
                    nc.vector.tensor_copy(k[mt][:, tsl], kps[:])
                    yield
                for st in range(hf * 4, hf * 4 + 4):
                    vps = ps.tile([P, C], F32, tag="mm", name="vps")
                    for kc in range(2):
                        nc.tensor.matmul(vps[:],
                                         lhsT=h[kc][:, st * P:(st + 1) * P],
                                         rhs=wv[l][:, kc, :],
                                         start=(kc == 0), stop=(kc == 1))
                    nc.gpsimd.memset(v[st][:, :, 32:33], 1.0)
                    nc.vector.tensor_copy(v[st][:, :, 0:32], vps[:])
                    yield

            def emit_remote_half(l, h, k, v, hf):
                """AllGather ONE t-half of h between the pair; k/v for the
                corresponding remote s-quarter (s 1024+512*hf ..+512).  The
                peer's B-half exists mid-layer, so gathering halves separately
                unblocks half the remote scores a whole post-chain earlier."""
                tsl = slice(hf * 512, (hf + 1) * 512)
                b_in = dp.tile([2 * P, 512], BF16, tag=f"b_in{hf}", name="b_in")
                b_out = dp.tile([4 * P, 512], BF16, tag=f"b_out{hf}",
                                name="b_out")
                for cc in range(2):
                    nc.sync.dma_start(b_in[cc * P:(cc + 1) * P, :],
                                      h[cc][:, tsl])
                if sim:
                    nc.sync.dma_start(b_out[:2 * P, :], b_in[:])
                    nc.sync.dma_start(b_out[2 * P:, :], b_in[:])
                else:
                    nc.gpsimd.collective_compute(
                        "AllGather", Alu.bypass, replica_groups=REPL,
                        ins=[b_in[:].opt()], outs=[b_out[:].opt()])
                hr = wk.tile([P, 2, 512], BF16, tag=f"hR{hf}", bufs=2, name="hR")
                nc.gpsimd.dma_gather(hr[:], b_out[:], remidx_sb[:], 2 * P, 2 * P,
                                     512)
                # spacing yields: the gather takes ~6us of DMA; emitting the
                # dependent matmuls right away would block the PE queue.
                for _ in range(6):
                    yield
                for mt in range(2):
                    kps = ps.tile([P, 512], F32, tag="mm", name="kps")
                    for kc in range(2):
                        nc.tensor.matmul(
                            kps[:], lhsT=wkt[l][:, kc, mt * P:(mt + 1) * P],
                            rhs=hr[:, kc, :],
                            start=(kc == 0), stop=(kc == 1))
                    nc.vector.tensor_copy(
                        k[mt][:, T + (hf - 2) * 512:T + (hf - 1) * 512], kps[:])
                    yield
                for st in range(8 + 4 * hf, 12 + 4 * hf):
                    vps = ps.tile([P, C], F32, tag="mm", name="vps")
                    for kc in range(2):
                        nc.tensor.matmul(
                            vps[:],
                            lhsT=hr[:, kc, (st - 8 - 4 * hf) * P:
                                  (st - 7 - 4 * hf) * P],
                            rhs=wv[l][:, kc, :],
                            start=(kc == 0), stop=(kc == 1))
                    nc.gpsimd.memset(v[st][:, :, 32:33], 1.0)
                    nc.vector.tensor_copy(v[st][:, :, 0:32], vps[:])
                    yield

            def alloc_attn_tiles():
                q = [wk.tile([P, TL], BF16, tag=f"qT{mt}", bufs=2,
                             name=f"qT{mt}") for mt in range(2)]
                k = [wk.tile([P, T], BF16, tag=f"kT{mt}", bufs=2,
                             name=f"kT{mt}") for mt in range(2)]
                v = [wk.tile([P, H, 33], BF16, tag=f"v{st}", bufs=2,
                             name=f"v{st}") for st in range(NS)]
                h = [wk.tile([P, TL], BF16, tag=f"hT{cc}", bufs=2,
                             name=f"hT{cc}") for cc in range(2)]
                return q, k, v, h

            # generator pump: emit a few chunks of deferred work per st slot.
            # Each generator carries a deadline (scheduling point by which its
            # instructions must have been emitted, because a later batch's
            # matmuls read its outputs and engines execute in emission order).
            pending = []   # list of [deadline, generator]

            def pump(n=1):
                for _ in range(n):
                    if not pending:
                        return
                    _MARKS.append((f"pump-d{pending[0][0]}", nc.next_id()))
                    try:
                        next(pending[0][1])
                    except StopIteration:
                        pending.pop(0)

            def drain_until(point):
                while pending and pending[0][0] <= point:
                    g = pending[0][1]
                    try:
                        while True:
                            next(g)
                    except StopIteration:
                        pending.pop(0)

            def drain():
                while pending:
                    pump()

            # ---------------- attention ----------------
            def attention_window(tt, o_acc, s0, s1, q, k, v, pump_n=1):
                for st in range(s0, s1):
                    S = ps.tile([P, H * P], F32, tag="S", name="S")
                    for j in range(H):
                        mt, jj = divmod(j, 4)
                        nc.tensor.matmul(
                            S[:, j * P:(j + 1) * P],
                            lhsT=k[mt][32 * jj:32 * (jj + 1),
                                       st * P:(st + 1) * P],
                            rhs=q[mt][32 * jj:32 * (jj + 1),
                                      tt * P:(tt + 1) * P],
                            start=True, stop=True, tile_position=(32 * jj, 0))
                    expS = ep.tile([P, H * P], BF16, tag="expT", name="expS")
                    nc.scalar.activation(expS[:], S[:], Act.Exp, scale=SCALE)
                    for j in range(H):
                        nc.tensor.matmul(o_acc[:, j, :],
                                         lhsT=expS[:, j * P:(j + 1) * P],
                                         rhs=v[st][:, j, :],
                                         start=(st == s0), stop=(st == s1 - 1),
                                         skip_group_check=True)
                    pump(pump_n)

            def attention_finish(tt, o_acc, p):
                if p is not None:
                    nc.vector.tensor_add(p[:], p[:], o_acc[:])
                    src = p
                else:
                    src = o_acc
                rec = sp.tile([P, H], F32, tag="rec", name="rec")
                nc.vector.reciprocal(rec[:], src[:, :, 32])
                o_sb = op.tile([P, C], BF16, tag="o_sb", name="o_sb")
                for j in range(H):
                    nc.vector.tensor_single_scalar(o_sb[:, j * HS:(j + 1) * HS],
                                                   src[:, j, 0:32],
                                                   rec[:, j:j + 1], Alu.mult)
                for cc in range(2):
                    nc.sync.dma_start_transpose(
                        oT_sb[:, cc, tt * P:(tt + 1) * P],
                        o_sb[:, cc * P:(cc + 1) * P])

            # ---------------- post-attention (per t-half) ----------------
            def post_half(l, hf, nxt):
                tsl = slice(hf * 512, (hf + 1) * 512)
                for cc in range(2):
                    dpj = ps.tile([P, 512], F32, tag="mm", name="dpj")
                    for kc in range(2):
                        nc.tensor.matmul(dpj[:],
                                         lhsT=wp[l][:, kc, cc * P:(cc + 1) * P],
                                         rhs=oT_sb[:, kc, tsl],
                                         start=(kc == 0), stop=(kc == 1))
                    nc.vector.scalar_tensor_tensor(xT[cc][:, tsl], dpj[:],
                                                   vap(l, 4, cc), xT[cc][:, tsl],
                                                   Alu.add, Alu.add)
                    yield
                yield from emit_ln_half(xT, 2 * l + 1,
                                        lambda cc: vap(l, 3, cc), h2T, hf)
                for ff in range(2):
                    fps = ps.tile([P, 512], F32, tag="mm", name="fps")
                    for kc in range(2):
                        nc.tensor.matmul(fps[:],
                                         lhsT=w1[l][:, kc, ff * P:(ff + 1) * P],
                                         rhs=h2T[kc][:, tsl],
                                         start=(kc == 0), stop=(kc == 1))
                    nc.vector.tensor_scalar(fT[ff][:, tsl], fps[:], vap(l, 5, ff),
                                            0.0, Alu.add, Alu.max)
                    yield
                for cc in range(2):
                    d2 = ps.tile([P, 512], F32, tag="mm", name="d2")
                    for kc in range(2):
                        nc.tensor.matmul(d2[:],
                                         lhsT=w2[l][:, kc, cc * P:(cc + 1) * P],
                                         rhs=fT[kc][:, tsl],
                                         start=(kc == 0), stop=(kc == 1))
                    nc.vector.scalar_tensor_tensor(xT[cc][:, tsl], d2[:],
                                                   vap(l, 6, cc), xT[cc][:, tsl],
                                                   Alu.add, Alu.add)
                    yield
                if l + 1 < L:
                    qn, kn, vn, hn = nxt
                    yield from emit_ln_half(xT, 2 * (l + 1),
                                            lambda cc: vap(l + 1, 1, cc), hn, hf)
                    yield from emit_qkv_local_half(l + 1, hf, hn, qn, kn, vn)
                else:
                    yield from emit_ln_half(xT, 2 * L,
                                            lambda cc: lnf_b_ap(cc),
                                            xf, hf)
                    for cc in range(2):
                        nc.vector.reduce_sum(emb2[:, hf, cc:cc + 1],
                                             xf[cc][:, tsl], axis=X_AXIS)
                    yield

            # ---------------- embedding + layer-0 prologue ----------------
            # bf16 token rows gathered straight into x^T layout (16-bit xbar
            # transpose in the gather DMA); fp32 residual formed by adding the
            # host-pretransposed fp32 positional embeddings.
            with tc.tile_pool(name="embed", bufs=1) as ebp:
                xg = ebp.tile([P, 2, TL], BF16, tag="xg")
                nc.gpsimd.dma_gather(xg[:], tok[:], idx_sb[:], TL, TL, C,
                                     transpose=True)
                pos_sb = ebp.tile([P, 2, TL], F32, tag="pos_sb")
                nc.sync.dma_start(pos_sb[:], posr[:])
                cur = alloc_attn_tiles()
                q0, k0, v0, h0 = cur
                for hf in (1, 0):   # B-half first: the opening wave needs it
                    tsl = slice(hf * 512, (hf + 1) * 512)
                    for cc in range(2):
                        nc.vector.tensor_add(xT[cc][:, tsl], xg[:, cc, tsl],
                                             pos_sb[:, cc, tsl])
                    for _ in emit_ln_half(xT, 0, lambda cc: vap(0, 1, cc),
                                          h0, hf):
                        pass
                    for _ in emit_qkv_local_half(0, hf, h0, q0, k0, v0):
                        pass
                pending.append([10, emit_remote_half(0, h0, k0, v0, 1)])
                pending.append([60, emit_remote_half(0, h0, k0, v0, 0)])

            # ---------------- transformer layers ----------------
            # Each layer processes the B-half t-tiles (4-7) FIRST so post-B
            # (which feeds the next layer's B tiles + the AllGather) runs
            # mid-layer, hidden under the A-tiles' exp stream; post-A runs at
            # the layer end and its serial chain is hidden by the next layer's
            # opening wave of (B-tile x s4-7) scores, which depend only on
            # post-B outputs.  Every tile runs in three s-windows with
            # partial-sum evacuation to SBUF, so 2 PSUM acc slots support a
            # 16-exp runway and all finishes read cheap SBUF partials.
            # Wave points: 0 = needs prior post-B only, 30 = needs post-A of
            # the previous layer (k/q A-half), 60 = needs this layer's remote.
            for l in range(L):
                q, k, v, h = cur
                nxt = alloc_attn_tiles() if l + 1 < L else None
                part = {}
                wincnt = {}
                TB, TA = (4, 5, 6, 7), (0, 1, 2, 3)
                sched = ([(tt, 4, 8, 0) for tt in TB]
                         + [(tt, 12, 16, 10) for tt in TB]
                         + [(tt, 0, 4, 30) for tt in TB]
                         + [(tt, 8, 12, 60) for tt in TB]
                         + [(tt, s0, s1, 60) for tt in TA
                            for s0, s1 in ((0, 4), (4, 8), (8, 16))])
                nwin = {tt: (4 if tt in TB else 3) for tt in range(NT)}
                for tt, s0, s1, pt in sched:
                    drain_until(l * 1000 + pt)
                    _MARKS.append((f"L{l}-tt{tt}-s{s0}", nc.next_id()))
                    o_acc = ps.tile([P, H, 33], F32, tag="acc",
                                    padded_shape=[P, H, 64], name="o_acc")
                    attention_window(tt, o_acc, s0, s1, q, k, v)
                    n = wincnt[tt] = wincnt.get(tt, 0) + 1
                    if n == 1:
                        p = wk.tile([P, H, 33], F32, tag=f"part{tt}", bufs=2,
                                    name=f"part{tt}")
                        nc.vector.tensor_copy(p[:], o_acc[:])
                        part[tt] = p
                    elif n < nwin[tt]:
                        nc.vector.tensor_add(part[tt][:], part[tt][:], o_acc[:])
                    else:
                        attention_finish(tt, o_acc, part.pop(tt))
                        if tt == 7:
                            pending.append([(l + 1) * 1000 + 0,
                                            post_half(l, 1, nxt)])
                            if l + 1 < L:
                                qn, kn, vn, hn = nxt
                                pending.append(
                                    [(l + 1) * 1000 + 10,
                                     emit_remote_half(l + 1, hn, kn, vn, 1)])
                        if tt == 3:
                            pending.append([(l + 1) * 1000 + 30,
                                            post_half(l, 0, nxt)])
                            if l + 1 < L:
                                qn, kn, vn, hn = nxt
                                pending.append(
                                    [(l + 1) * 1000 + 60,
                                     emit_remote_half(l + 1, hn, kn, vn, 0)])
                cur = nxt
            drain()

            # ---------------- pool + classifier ----------------
            emb = sp.tile([P, 2], F32, tag="emb", bufs=1)
            nc.vector.tensor_add(emb[:], emb2[:, 0, :], emb2[:, 1, :])
            be_in = dp.tile([P, 2], F32, tag="be_in", name="be_in")
            be_out = dp.tile([P, 2], F32, tag="be_out", name="be_out")
            nc.sync.dma_start(be_in[:], emb[:])
            if sim:
                nc.sync.dma_start(be_out[:], be_in[:])
            else:
                nc.gpsimd.collective_compute(
                    "AllReduce", Alu.add, replica_groups=REPL,
                    ins=[be_in[:].opt()], outs=[be_out[:].opt()])
            embr = sp.tile([P, 2], F32, tag="embr", bufs=1)
            nc.sync.dma_start(embr[:], be_out[:])

            h1ps = ps.tile([P, CLS_H // P], F32, tag="mm", name="h1ps")
            for mt in range(CLS_H // P):
                for kc in range(2):
                    nc.tensor.matmul(h1ps[:, mt:mt + 1],
                                     lhsT=wc1_ap(kc, mt),
                                     rhs=embr[:, kc:kc + 1],
                                     start=(kc == 0), stop=(kc == 1))
            h1 = sp.tile([P, CLS_H // P], F32, tag="h1", bufs=1)
            nc.vector.tensor_add(h1[:], h1ps[:], bc1_ap)
            nc.vector.tensor_scalar_max(h1[:], h1[:], 0.0)
            lps = ps.tile([1, NOUT], F32, tag="mm", name="lps")
            for j in range(CLS_H // P):
                nc.tensor.matmul(lps[:], lhsT=h1[:, j:j + 1], rhs=wc2_ap(j),
                                 start=(j == 0), stop=(j == CLS_H // P - 1))
            lsb = sp.tile([1, NOUT], F32, tag="lsb", bufs=1)
            nc.vector.tensor_add(lsb[:], lps[:], bc2_ap)
            mx = sp.tile([1, 1], F32, tag="mx", bufs=1)
            nc.vector.tensor_reduce(mx[:], lsb[:], axis=X_AXIS, op=Alu.max)
            nmx = sp.tile([1, 1], F32, tag="nmx", bufs=1)
            nc.vector.tensor_scalar_mul(nmx[:], mx[:], -1.0)
            esb = sp.tile([1, NOUT], F32, tag="esb", bufs=1)
            nc.scalar.activation(esb[:], lsb[:], Act.Exp, bias=nmx[:])
            ssum = sp.tile([1, 1], F32, tag="ssum", bufs=1)
            nc.vector.reduce_sum(ssum[:], esb[:], axis=X_AXIS)
            rsum = sp.tile([1, 1], F32, tag="rsum", bufs=1)
            nc.vector.reciprocal(rsum[:], ssum[:])
            probs = sp.tile([1, NOUT], F32, tag="probs", bufs=1)
            nc.vector.tensor_single_scalar(probs[:], esb[:], rsum[:], Alu.mult)
            nc.sync.dma_start(out_d[:], probs[:])

    nc.compile()
    return nc


def _prep_shared(inputs):
    """Host-side weight prepack (identical for all cores)."""
    f = lambda a: np.ascontiguousarray(np.asarray(a, dtype=np.float32))

    def pack_mat(w, dtyp=BF):  # [C_in, M] -> [128, C_in//128, M]
        ci, m = w.shape
        return np.ascontiguousarray(
            w.reshape(ci // P, P, m).transpose(1, 0, 2).astype(dtyp))

    wq3 = np.stack([pack_mat(f(inputs["Wq"][l]).transpose(1, 0, 2).reshape(C, H * HS))
                    for l in range(L)])
    wk3 = np.stack([pack_mat(f(inputs["Wk"][l]).transpose(1, 0, 2).reshape(C, H * HS))
                    for l in range(L)])
    wv3 = np.stack([pack_mat(f(inputs["Wv"][l]).transpose(1, 0, 2).reshape(C, H * HS))
                    for l in range(L)])
    wp3 = np.stack([pack_mat(f(inputs["Wproj"][l])) for l in range(L)])
    w13 = np.stack([pack_mat(f(inputs["W1"][l])) for l in range(L)])
    w23 = np.stack([pack_mat(f(inputs["W2"][l])) for l in range(L)])
    # [6, L, P, 2, C] -> [P, 6, L, 2, C]
    wsb = np.ascontiguousarray(
        np.stack([wq3, wk3, wv3, wp3, w13, w23]).transpose(2, 0, 1, 3, 4))

    def pack_vec(v):  # [256] -> [128, 2]
        return np.ascontiguousarray(f(v).reshape(2, P).T)

    vecs = np.stack([np.stack([pack_vec(inputs[k][l]) for k in
                               ("ln1_g", "ln1_b", "ln2_g", "ln2_b",
                                "bproj", "b1", "b2")]).transpose(1, 0, 2)
                     for l in range(L)])          # [L, P, 7, 2]
    vecs = np.ascontiguousarray(vecs.transpose(1, 0, 2, 3))   # [P, L, 7, 2]
    gt_rows = []
    for l in range(L):
        gt_rows.append(f(inputs["ln1_g"][l]))
        gt_rows.append(f(inputs["ln2_g"][l]))
    gt_rows.append(f(inputs["lnf_g"]))
    gtc = np.concatenate([np.stack(gt_rows).reshape(-1),
                          f(inputs["bc2"]).reshape(-1)])[None]  # [1, 9C+10]
    wc1 = pack_mat(f(inputs["Wc1"]) / T, np.float32)  # fold mean-pool 1/T
    bc1 = np.ascontiguousarray(f(inputs["bc1"]).reshape(CLS_H // P, P).T)
    wc2 = np.ascontiguousarray(f(inputs["Wc2"]).reshape(CLS_H // P, P, NOUT)
                               .transpose(1, 0, 2))
    lnfv = np.stack([pack_vec(inputs["lnf_g"]),
                     pack_vec(inputs["lnf_b"])])    # [2(g,b), P, 2(cc)]
    csb = np.concatenate([wc1.reshape(P, -1), wc2.reshape(P, -1), bc1,
                          lnfv.transpose(1, 0, 2).reshape(P, 4)], axis=1)
    tokf = np.ascontiguousarray(f(inputs["tok_emb"]).astype(BF))
    posf = f(inputs["pos_emb"])
    return dict(wsb=wsb, vecs=vecs, gtc=np.ascontiguousarray(gtc),
                csb=np.ascontiguousarray(csb), tok=tokf, pos=posf)


def _wrap_idx(ids):
    """int array [n] -> dma_gather wrapped layout [128, n//16] int16."""
    n = ids.shape[0]
    w = ids.reshape(n // 16, 16).T.astype(np.int16)     # [16, n//16]
    return np.ascontiguousarray(np.tile(w, (8, 1)))     # [128, n//16]


def _make_in_maps(inputs):
    shared = _prep_shared(inputs)
    idx = np.asarray(inputs["idx"]).astype(np.int64)
    in_maps = []
    for c in range(N_CORES):
        b, th = c // 2, c % 2
        t0 = th * TL
        idx_loc = idx[b, t0:t0 + TL]
        pos_loc = shared["pos"][t0:t0 + TL]  # [TL, C]
        # pos^T chunks: posr[p, cc, t] = pos[t, cc*128 + p]
        posr_a = np.ascontiguousarray(
            pos_loc.T.reshape(2, P, TL).transpose(1, 0, 2))
        rem = (1 - th) * 2 * P + np.arange(2 * P, dtype=np.int64)
        m = dict(tok=shared["tok"], idxw=_wrap_idx(idx_loc), posr=posr_a,
                 remidx=_wrap_idx(rem), wsb=shared["wsb"],
                 vecs=shared["vecs"], gtc=shared["gtc"], csb=shared["csb"])
        in_maps.append(m)
    return in_maps


def kernel(**inputs) -> np.ndarray:
    if "nc" not in _CACHE:
        _CACHE["nc"] = _build_program()
    nc = _CACHE["nc"]
    in_maps = _make_in_maps(inputs)
    res = bass_utils.run_bass_kernel_spmd(nc, in_maps, core_ids=list(range(N_CORES)))
    out = np.zeros((B, NOUT), np.float32)
    for b in range(B):
        out[b] = res.results[2 * b]["probs"][0]
    return out
